# revision 40
# baseline (speedup 1.0000x reference)
"""Multi-head self-attention Trainium2 kernel (8 NeuronCores, SPMD).

Problem: B=2, N=4096, D=512, H=8 heads of dim 64.
  qkv = x @ qkv_w.T + qkv_b ; per-head attention with softmax(QK^T/8) ;
  out = attn @ out_w.T + out_b

Sharding: 16 (batch, head) pairs -> 8 cores, each core owns one batch b and
one head-PAIR (2 adjacent heads = a 128-row slice of the qkv projections).
Each core computes the full attention for its 2 heads over all 4096 rows and
a partial output projection; the host sums the 4 per-batch partials and adds
the (folded) biases.

On-chip layout strategy: everything is computed with the contraction dim on
partitions so no transposes are ever needed:
  Q^T,K^T [128d, 4096]  <- lhsT=W^T tiles, rhs=x^T
  V       [4096, 128d]  (natural; lhsT=x^T tile, rhs=Wv^T) + fused ones column
  S^T = K^T-stationary matmul, 2 heads row-packed (K=64 each) in the PE array
  P^T = exp(S^T) straight out of PSUM (no max-subtraction: |S|<~3).
  exp is split across THREE engines: most tiles on ScalarE
  (activation Exp with scale=4; S is pre-scaled by 1/4 in the weights, an
  exact power-of-2 fold), the rest via a custom DVE op computing a Horner
  deg-4 polynomial q~exp(u) on u=S/4 (1 pass) followed by two squaring
  passes on the otherwise-idle GPSIMD engine: P = ((q)^2)^2 = exp(S).
  O^T accum = (V|1)-stationary matmul over P^T; row 64 = softmax denominator
  normalize via reciprocal_approx_fast + PE outer-product broadcast; partial
  y^T = Wout^T slice-stationary matmul.
All biases are folded on the host.
"""

import os
import numpy as np
import ml_dtypes

B, N, D, H, HD = 2, 4096, 512, 8, 64
NCORES = 8
KT_TILES = 4      # D / 128 contraction tiles
JT = 32           # N / 128 key tiles
ICH = 8           # N / 512 query chunks
P = 128

MODE = os.environ.get("ATTN_KERNEL_MODE", "bf16")

# exp-poly coefficients: p(u) = 1 + u + C2 u^2 + C3 u^3 + C4 u^4 ~ exp(u)
# on [-0.8, 0.8]; P = p^4 ~ exp(4u), max rel err 1.3e-3.
EC2, EC3, EC4 = 0.50133404, 0.17126203, 0.03980059

# which jt positions (per ic chunk) take the DVE+GPSIMD exp route.
# Chosen clear of the norm-quanta DVE work (jts 2-7) and so every DVE PV
# (jt + DVE_LAG) lands within the same ic -> the ic's LAST issued PV stays
# jt31's (scalar, next-ic jt0) and finalize timing matches the o-pool
# rotation.  The last ic routes only the early ones to shorten the drain.
DVE_JTS = frozenset((0, 8, 12, 16, 20, 24, 28))
DVE_JTS_LAST = frozenset((0, 8, 12, 16, 20))
DVE_LAG = 4       # iterations between S-matmul and PV h0 issue (h1 lags +1)
SC_LAG = 1        # same for ScalarE-route tiles (slack against engine jitter)

_BUILD_CACHE = {}
_EXP_OP = None


def _register_exp_op():
    """Register the Horner deg-4 custom DVE op (idempotent)."""
    global _EXP_OP
    if _EXP_OP is not None:
        return _EXP_OP
    from concourse import dve_ops
    from concourse.dve_spec import Spec, Src0, C0, C1, C2, One, lower, _has_src1
    from concourse.dve_uop import DveOpSpec

    name = "EXP_P4_ANT"
    if name in dve_ops._SUB_OPCODE_FOR_NAME:
        _EXP_OP = next(o for o in dve_ops.OPS if o.name == name)
        return _EXP_OP
    body = (((C2 * Src0 + C1) * Src0 + C0) * Src0 + One) * Src0 + One
    spec = Spec(
        body=body,
        reference=lambda in0, in1, s0, s1, imm2: (
            (((imm2 * in0 + s1) * in0 + s0) * in0 + 1.0) * in0 + 1.0
        ).astype(np.float32),
    )
    row = max(dve_ops._SUB_OPCODE_FOR_NAME.values()) + 1
    dve_ops._SUB_OPCODE_FOR_NAME[name] = row
    tmp = DveOpSpec(name=name, opcode=row, uops=lower(spec, ver="v3"),
                    rd1_en=_has_src1(spec))
    op = dve_ops.DveOp(name, spec, subdim=False, uops_sha={"v3": tmp.sha("v3")})
    dve_ops.OPS.append(op)
    dve_ops.CUSTOM_DVE_SPECS[name] = spec
    _EXP_OP = op
    return op


def _np_dt(dt):
    import concourse.mybir as mybir
    return np.dtype(ml_dtypes.bfloat16) if dt == mybir.dt.bfloat16 else np.dtype(np.float32)


def _build(mode):
    """Build (and cache) the compiled Bass program for all cores (SPMD)."""
    if mode in _BUILD_CACHE:
        return _BUILD_CACHE[mode]

    import concourse.bacc as bacc
    import concourse.mybir as mybir
    import concourse.tile as tile
    from concourse.bass import _add_dep_helper
    from contextlib import ExitStack

    exp_op = _register_exp_op()

    f32 = mybir.dt.float32
    bf16 = mybir.dt.bfloat16
    if mode == "bf16":
        dt_qk, dt_pv = bf16, bf16
    elif mode == "mixed":
        dt_qk, dt_pv = f32, bf16
    else:
        dt_qk, dt_pv = f32, f32

    Exp = mybir.ActivationFunctionType.Exp
    mult = mybir.AluOpType.mult

    nc = bacc.Bacc(None, target_bir_lowering=False)
    xt_d = nc.dram_tensor("xt", [KT_TILES, P, N], dt_qk, kind="ExternalInput")
    wqt_d = nc.dram_tensor("wqt", [KT_TILES, P, P], dt_qk, kind="ExternalInput")
    wkt_d = nc.dram_tensor("wkt", [KT_TILES, P, P], dt_qk, kind="ExternalInput")
    wvt_d = nc.dram_tensor("wvt", [KT_TILES, P, P], dt_qk, kind="ExternalInput")
    wot_d = nc.dram_tensor("wot", [2, HD, D], dt_pv, kind="ExternalInput")
    bq_d = nc.dram_tensor("bq", [P, 1], f32, kind="ExternalInput")
    bk_d = nc.dram_tensor("bk", [P, 1], f32, kind="ExternalInput")
    yp_d = nc.dram_tensor("yp", [KT_TILES, P, N], f32, kind="ExternalOutput")

    def ics(i):
        return slice(i * 512, (i + 1) * 512)

    def jts(j):
        return slice(j * P, (j + 1) * P)

    def mts(m):
        return slice(m * P, (m + 1) * P)

    with tile.TileContext(nc) as tc, ExitStack() as ctx:
        const = ctx.enter_context(tc.tile_pool(name="const", bufs=1))
        sp = ctx.enter_context(tc.tile_pool(name="spool", bufs=2, space="PSUM"))
        op = ctx.enter_context(tc.tile_pool(name="opool", bufs=3, space="PSUM"))
        mp = ctx.enter_context(tc.tile_pool(name="mpool", bufs=1, space="PSUM"))
        pp = ctx.enter_context(tc.tile_pool(name="ppool", bufs=10))
        yep = ctx.enter_context(tc.tile_pool(name="yepool", bufs=3))
        rrp = ctx.enter_context(tc.tile_pool(name="rrpool", bufs=2))
        rbp = ctx.enter_context(tc.tile_pool(name="rbpool", bufs=2))
        drp = ctx.enter_context(tc.tile_pool(name="drpool", bufs=2))
        eyp = ctx.enter_context(tc.tile_pool(name="eypool", bufs=4))
        ezp = ctx.enter_context(tc.tile_pool(name="ezpool", bufs=4))

        xt = const.tile([P, KT_TILES, N], dt_qk, tag="xt")
        wqt = const.tile([P, KT_TILES, P], dt_qk, tag="wqt")
        wkt = const.tile([P, KT_TILES, P], dt_qk, tag="wkt")
        wvt = const.tile([P, KT_TILES, P], dt_qk, tag="wvt")
        # weight DMAs spread over three queues, K first (the kproj chains
        # gate the first S matmul), V/O last (not needed until jt0 of ic0)
        bq = const.tile([P, 1], f32, tag="bq")
        bk = const.tile([P, 1], f32, tag="bk")
        nc.scalar.dma_start(bq[:], bq_d[:])
        nc.scalar.dma_start(bk[:], bk_d[:])
        for k in range(KT_TILES):
            q = nc.gpsimd if k < 2 else nc.scalar
            q.dma_start(wkt[:, k, :], wkt_d[k])
        for k in range(KT_TILES):
            q = nc.gpsimd if k < 2 else nc.scalar
            q.dma_start(wqt[:, k, :], wqt_d[k])
        for k in range(KT_TILES):
            nc.gpsimd.dma_start(wvt[:, k, :], wvt_d[k])
        # x^T in column-major chunk order on ONE queue: the first column
        # blocks (all k-tiles) land at ~12% of the transfer, so the Q/K
        # projections and early attention start ~7us sooner than waiting for
        # whole k-tiles (total landing time is HBM-stack-BW-bound either way)
        XCH = 512
        for c in range(N // XCH):
            for k in range(KT_TILES):
                nc.sync.dma_start(xt[:, k, c * XCH:(c + 1) * XCH],
                                  xt_d[k][:, c * XCH:(c + 1) * XCH])
        wot = const.tile([HD, 2, D], dt_pv, tag="wot")
        for h in range(2):
            nc.scalar.dma_start(wot[:, h, :], wot_d[h])

        QT = const.tile([P, N], dt_qk, tag="QT")
        KT = const.tile([P, N], dt_qk, tag="KT")
        Vp = const.tile([P, JT, 130], dt_pv, tag="Vp")
        OT0 = const.tile([HD, N], dt_pv, tag="OT0")
        OT1 = const.tile([HD, N], dt_pv, tag="OT1")
        ones = const.tile([65, HD], dt_pv, tag="ones")
        nc.vector.memset(ones[64:65, :], 1.0)
        actwarm = const.tile([1, 1], f32, tag="actwarm")
        nc.vector.memset(actwarm[:], 0.0)
        nc.scalar.activation(actwarm[:], actwarm[:], Exp)
        # warm up gpsimd + DVE custom path before the steady loop
        gpw = const.tile([1, 8], f32, tag="gpw")
        nc.vector.memset(gpw[:], 1.0)
        nc.gpsimd.tensor_mul(gpw[:], gpw[:], gpw[:])
        dvw = const.tile([1, 8], f32, tag="dvw")
        nc.vector.memset(dvw[:], 0.0)
        nc.vector._custom_dve(exp_op, out=dvw[:], in0=dvw[:],
                              s0=EC2, s1=EC3, imm2=EC4)
        nc.vector.memset(Vp[:, :, 64:65], 1.0)
        nc.vector.memset(Vp[:, :, 129:130], 1.0)

        # ---- projection units (emitted interleaved into the attention loop
        # so the PE prefix before the first exp is tiny) ----
        def qproj_unit(ic):
            # Q^T[:, ic] (uses the otherwise-idle mp psum bank)
            ps = mp.tile([P, 512], f32, tag="mp", name=f"qp_{ic}")
            for k in range(KT_TILES):
                nc.tensor.matmul(ps[:], wqt[:, k, :], xt[:, k, ics(ic)],
                                 start=(k == 0), stop=(k == KT_TILES - 1))
            nc.vector.tensor_scalar_add(QT[:, ics(ic)], ps[:], bq[:, 0:1])

        def qproj_half_d(ic, half, after=None):
            # one [128,256] half of the deferred Q^T projection chain
            qs = slice(ics(ic).start + 256 * half, ics(ic).start + 256 * (half + 1))
            ps = mp.tile([P, 256], f32, tag="mp", name=f"qpd_{ic}_{half}")
            for k in range(KT_TILES):
                mm = nc.tensor.matmul(ps[:], wqt[:, k, :], xt[:, k, qs],
                                      start=(k == 0), stop=(k == KT_TILES - 1))
                if after is not None and k == 0:
                    _add_dep_helper(mm.ins, after.ins, sync=False,
                                    reason="defer qproj behind attention")
            nc.vector.tensor_scalar_add(QT[:, qs], ps[:], bq[:, 0:1])

        def kproj_unit(jc):
            # K^T[:, jc*512:(jc+1)*512]
            ps = mp.tile([P, 512], f32, tag="mp", name=f"kp_{jc}")
            for k in range(KT_TILES):
                nc.tensor.matmul(ps[:], wkt[:, k, :], xt[:, k, ics(jc)],
                                 start=(k == 0), stop=(k == KT_TILES - 1))
            nc.vector.tensor_scalar_add(KT[:, ics(jc)], ps[:], bk[:, 0:1])

        def vproj_unit(jt):
            # V[jt] (natural layout) + split into the two per-head Vp slabs
            ps = op.tile([P, P], f32, tag="o", name=f"vp_{jt}")
            for k in range(KT_TILES):
                nc.tensor.matmul(ps[:], xt[:, k, jts(jt)], wvt[:, k, :],
                                 start=(k == 0), stop=(k == KT_TILES - 1))
            nc.vector.tensor_copy(Vp[:, jt, 0:64], ps[:, 0:64])
            nc.vector.tensor_copy(Vp[:, jt, 65:129], ps[:, 64:128])

        def kproj_unit_s(jc):
            # K^T chunk on an s-pool slot (prefix only: runs parallel to the
            # qproj on the mp bank)
            ps = sp.tile([P, 512], f32, tag="s", name=f"kps_{jc}")
            for k in range(KT_TILES):
                nc.tensor.matmul(ps[:], wkt[:, k, :], xt[:, k, ics(jc)],
                                 start=(k == 0), stop=(k == KT_TILES - 1))
            nc.vector.tensor_scalar_add(KT[:, ics(jc)], ps[:], bk[:, 0:1])

        # upfront: Q chunk 0 on mp, K chunks 0+1 on the two s-pool slots
        qproj_unit(0)
        kproj_unit_s(0)
        kproj_unit_s(1)

        # ---- attention (software-pipelined emission: S/exp of step t, PV of
        # ready pending steps; ScalarE tiles are PV-ready after 1 iteration,
        # DVE-route tiles after DVE_LAG (their P goes DVE poly -> gpsimd
        # square -> gpsimd square).  finalize is staged as in the baseline:
        # DVE-only work right after the last PV; PE work several iterations
        # later so the PE FIFO never waits on the reciprocal. ----
        def finalize_a(ic, o0, o1):
            # PSUM evict + bf16 cast of the denominator rows (row 64); the
            # bf16 rows make the rb broadcast matmuls run at bf16 rate
            # (fp32 matmuls are ~5x more PE time).  In the drain tail the
            # h1 copies go to the otherwise-idle ScalarE so the four copies
            # run two-abreast.
            oss = []
            for i, o in enumerate((o0, o1)):
                os_ = rrp.tile([65, 512], f32, tag="os", name=f"os_{ic}_{i}")
                dr = drp.tile([65, 512], dt_pv, tag="dr", name=f"dr_{ic}_{i}")
                if in_tail and i == 1:
                    nc.scalar.copy(os_[:], o[:])
                    nc.scalar.copy(dr[64:65, :], o[64:65, :])
                else:
                    nc.vector.tensor_copy(os_[:], o[:])
                    nc.vector.tensor_copy(dr[64:65, :], o[64:65, :])
                oss.append((os_, dr))
            return oss

        def norm_quantum(ic, st, h, half, after=None):
            # normalize one head/half: OT[:, slice] = os[0:64] * (1/r) via PE
            # outer-product of the (bf16) denominator (no recip dep in PE
            # FIFO) then reciprocal+mul on DVE.
            os_, dr = st[h]
            OTt = (OT0, OT1)[h]
            ls = slice(256 * half, 256 * (half + 1))
            hs = slice(ics(ic).start + 256 * half, ics(ic).start + 256 * (half + 1))
            pool_h = mp if h == 0 else op
            rb = pool_h.tile([HD, 256], f32, tag="mp" if h == 0 else "o",
                             name=f"rb_{ic}_{h}_{half}")
            mm = nc.tensor.matmul(rb[:], ones[64:65, :], dr[64:65, ls],
                                  start=True, stop=True, tile_position=(64, 0))
            if after is not None:
                _add_dep_helper(mm.ins, after.ins, sync=False,
                                reason="defer finalize rb behind attention")
            rbs = rbp.tile([HD, 256], f32, tag="rbs", name=f"rbs_{ic}_{h}_{half}")
            nc.vector.reciprocal_approx_fast(out=rbs[:], in_=rb[:])
            nc.vector.tensor_mul(OTt[:, hs], os_[0:64, ls], rbs[:])

        def yproj_quantum(ic, mt, idx, after=None, tail=False):
            # one [128,512] slice of the partial output projection; tail
            # quanta alternate between the mp and (now idle) o-pool banks
            hs = ics(ic)
            pool_y = op if (tail and mt % 2 == 1) else mp
            yps = pool_y.tile([P, 512], f32,
                              tag="o" if (tail and mt % 2 == 1) else "mp",
                              name=f"yp_{ic}_{mt}")
            mm = nc.tensor.matmul(yps[:], wot[:, 0, mts(mt)], OT0[:, hs],
                                  start=True, stop=False)
            if after is not None:
                _add_dep_helper(mm.ins, after.ins, sync=False,
                                reason="defer finalize yproj behind attention")
            nc.tensor.matmul(yps[:], wot[:, 1, mts(mt)], OT1[:, hs],
                             start=False, stop=True)
            ye = yep.tile([P, 512], f32, tag="ye", name=f"ye_{ic}_{mt}")
            if tail:
                nc.scalar.copy(ye[:], yps[:])   # ScalarE is idle in the tail
            else:
                nc.vector.tensor_copy(ye[:], yps[:])
            nc.sync.dma_start(yp_d[mt, :, hs], ye[:])

        # quantum schedule within the NEXT chunk: (jt, fn(args))
        NORM_SCHED = [(2, (0, 0)), (4, (1, 0)), (5, (0, 1)), (7, (1, 1))]
        YP_SCHED = {9: 0, 11: 1, 13: 2, 15: 3}

        def emit_exp(s, ic, jt):
            """Emit the exp of s -> P tile; returns (p_tile, ready_lag).

            DVE route runs per 512-col half (one head each) so the
            DVE->gpsimd->gpsimd chain latency per consumed half is short."""
            p = pp.tile([P, 1024], dt_pv, tag="p")
            route = DVE_JTS_LAST if ic == ICH - 1 else DVE_JTS
            if jt in route:
                # per-half chains: gpsimd starts squaring h0 while the DVE
                # polys h1, so P-h0 lands well before its lag-4 PV deadline
                for h in range(2):
                    cs = slice(512 * h, 512 * (h + 1))
                    ey = eyp.tile([P, 512], f32, tag="ey", name=f"ey_{ic}_{jt}_{h}")
                    nc.vector._custom_dve(exp_op, out=ey[:], in0=s[:, cs],
                                          s0=EC2, s1=EC3, imm2=EC4)
                    ez = ezp.tile([P, 512], f32, tag="ez", name=f"ez_{ic}_{jt}_{h}")
                    nc.gpsimd.tensor_mul(ez[:], ey[:], ey[:])
                    nc.gpsimd.tensor_tensor(p[:, cs], ez[:], ez[:], mult)
                return p, DVE_LAG, True
            nc.scalar.activation(p[:], s[:], Exp, scale=4.0)
            return p, SC_LAG, False

        otiles = {}
        in_tail = False
        issued0 = {}         # ic -> PV h0 matmuls issued
        issued1 = {}         # ic -> PV h1 matmuls issued
        pend = []            # list of [p_tile, ic, jt, ready_g]
        pend_h1 = []         # deferred first-of-ic h1 halves (o-bank WAR slack)
        pend_b = None        # (ic, stage-a state) awaiting finalize_b
        g = 0

        def issue_pv_h1(ptile, pic, pjt):
            nonlocal pend_b
            o0, o1 = otiles[pic]
            nc.tensor.matmul(o1[:], Vp[:, pjt, 65:130], ptile[:, 512:1024],
                             start=(issued1[pic] == 0),
                             stop=(issued1[pic] == JT - 1))
            issued1[pic] += 1
            if issued0[pic] == JT and issued1[pic] == JT:
                pend_b = (pic, finalize_a(pic, o0, o1))

        def issue_pv(ptile, pic, pjt, defer_h1=False):
            # h0 immediately; h1 deferred one iteration for the ic's FIRST
            # pair (o-bank WAR slack vs the previous ic's o0 eviction) and
            # for DVE-route tiles (their h1 half lands one iteration later)
            o0, o1 = otiles[pic]
            nc.tensor.matmul(o0[:], Vp[:, pjt, 0:65], ptile[:, 0:512],
                             start=(issued0[pic] == 0),
                             stop=(issued0[pic] == JT - 1))
            issued0[pic] += 1
            if defer_h1 or (issued1[pic] == 0 and issued0[pic] == 1):
                pend_h1.append((ptile, pic, pjt))
            else:
                issue_pv_h1(ptile, pic, pjt)

        for ic in range(ICH):
            otiles[ic] = (op.tile([65, 512], f32, tag="o", name=f"o0_{ic}"),
                          op.tile([65, 512], f32, tag="o", name=f"o1_{ic}"))
            issued0[ic] = 0
            issued1[ic] = 0
            for jt in range(JT):
                s = sp.tile([P, 1024], f32, tag="s")
                last_s = nc.tensor.matmul(s[:, 0:512], KT[0:64, jts(jt)],
                                          QT[0:64, ics(ic)],
                                          start=True, stop=True,
                                          tile_position=(0, 0))
                nc.tensor.matmul(s[:, 512:1024], KT[64:128, jts(jt)],
                                 QT[64:128, ics(ic)],
                                 start=True, stop=True, tile_position=(64, 0))
                p, lag, isdve = emit_exp(s, ic, jt)
                pend.append([p, ic, jt, g + lag, isdve])
                # deferred h1 halves first, then all PVs ready at this point
                # (issue order is the PSUM accumulation order)
                for ent in pend_h1[:]:
                    pend_h1.remove(ent)
                    issue_pv_h1(*ent)
                for ent in [e for e in pend if e[3] <= g]:
                    pend.remove(ent)
                    issue_pv(ent[0], ent[1], ent[2], defer_h1=ent[4])
                # deferred proj + finalize stages, spread across the loop
                if ic == 0:
                    if jt == 0:
                        vproj_unit(0)
                        vproj_unit(1)
                    elif jt <= JT - 2:
                        vproj_unit(jt + 1)
                    if jt < 24 and jt % 4 == 0:
                        kproj_unit(2 + jt // 4)
                if pend_b is not None:
                    bic, st = pend_b
                    for sjt, (h, half) in NORM_SCHED:
                        if jt == sjt:
                            norm_quantum(bic, st, h, half, after=last_s)
                    if jt in YP_SCHED:
                        mt = YP_SCHED[jt]
                        yproj_quantum(bic, mt, mt, after=last_s)
                        if mt == 3:
                            pend_b = None
                if jt in (19, 21) and ic + 1 < ICH:
                    qproj_half_d(ic + 1, (jt - 19) // 2, after=last_s)
                g += 1
        # drain the pipeline tail
        in_tail = True
        for ent in pend_h1[:]:
            pend_h1.remove(ent)
            issue_pv_h1(*ent)
        for ent in sorted(pend, key=lambda e: e[3]):
            issue_pv(ent[0], ent[1], ent[2])
        pic, st = pend_b
        for _, (h, half) in NORM_SCHED:
            norm_quantum(pic, st, h, half)
        for mt in range(4):
            yproj_quantum(pic, mt, mt, tail=True)

    nc.compile()
    _BUILD_CACHE[mode] = nc
    return nc


def _prep_inputs(x, qkv_w, qkv_b, out_w, mode):
    """Per-core input maps. Core c: batch c//4, head-pair c%4."""
    if mode == "bf16":
        dt_qk = np.dtype(ml_dtypes.bfloat16)
        dt_pv = dt_qk
    elif mode == "mixed":
        dt_qk = np.dtype(np.float32)
        dt_pv = np.dtype(ml_dtypes.bfloat16)
    else:
        dt_qk = np.dtype(np.float32)
        dt_pv = dt_qk

    x = np.asarray(x, np.float32)
    qkv_w = np.asarray(qkv_w, np.float32)
    qkv_b = np.asarray(qkv_b, np.float32)
    out_w = np.asarray(out_w, np.float32)

    xts = []
    for b in range(B):
        xt = np.ascontiguousarray(x[b].T).reshape(KT_TILES, P, N)
        xts.append(xt.astype(dt_qk))

    # q scale: 1/sqrt(hd)=0.125 plus an extra exact 1/4 so PSUM holds S/4
    # (ScalarE exp uses scale=4; the DVE poly route consumes S/4 directly)
    qs = 0.125 * 0.25
    in_maps = []
    for c in range(NCORES):
        b, m = divmod(c, 4)
        rs = slice(P * m, P * (m + 1))
        wq = (qs * qkv_w[0:D][rs]).T.reshape(KT_TILES, P, P)
        wk = qkv_w[D:2 * D][rs].T.reshape(KT_TILES, P, P)
        wv = qkv_w[2 * D:3 * D][rs].T.reshape(KT_TILES, P, P)
        wo = np.ascontiguousarray(out_w[:, rs].T).reshape(2, HD, D)
        in_maps.append({
            "xt": xts[b],
            "wqt": np.ascontiguousarray(wq).astype(dt_qk),
            "wkt": np.ascontiguousarray(wk).astype(dt_qk),
            "wvt": np.ascontiguousarray(wv).astype(dt_qk),
            "wot": wo.astype(dt_pv),
            "bq": (qs * qkv_b[0:D][rs]).reshape(P, 1).astype(np.float32),
            "bk": qkv_b[D:2 * D][rs].reshape(P, 1).astype(np.float32),
        })
    return in_maps


def _gather(results, qkv_b, out_w, out_b):
    # y[b] = (sum over the batch's 4 cores of yp)^T + out_w @ bv + out_b
    bias_vec = out_w.astype(np.float32) @ np.asarray(qkv_b, np.float32)[2 * D:3 * D] \
        + np.asarray(out_b, np.float32)
    y = np.empty((B, N, D), np.float32)
    for b in range(B):
        acc = np.zeros((D, N), np.float32)
        for m in range(4):
            acc += results[4 * b + m]["yp"].reshape(D, N)
        y[b] = acc.T + bias_vec
    return y


def _run(inputs, trace=False, tmpdir=None):
    from concourse.bass_utils import run_bass_kernel_spmd

    nc = _build(MODE)
    in_maps = _prep_inputs(inputs["x"], inputs["qkv_w"], inputs["qkv_b"],
                           inputs["out_w"], MODE)
    kw = {}
    if trace:
        kw = dict(trace=True, tmpdir=tmpdir)
    res = run_bass_kernel_spmd(nc, in_maps, core_ids=list(range(NCORES)), **kw)
    y = _gather(res.results, inputs["qkv_b"], inputs["out_w"], inputs["out_b"])
    return y, res


def kernel(x, qkv_w, qkv_b, out_w, out_b):
    y, _ = _run(dict(x=x, qkv_w=qkv_w, qkv_b=qkv_b, out_w=out_w, out_b=out_b))
    return y


# revision 41
# speedup vs baseline: 1.0015x; 1.0015x over previous
"""Multi-head self-attention Trainium2 kernel (8 NeuronCores, SPMD).

Problem: B=2, N=4096, D=512, H=8 heads of dim 64.
  qkv = x @ qkv_w.T + qkv_b ; per-head attention with softmax(QK^T/8) ;
  out = attn @ out_w.T + out_b

Sharding: 16 (batch, head) pairs -> 8 cores, each core owns one batch b and
one head-PAIR (2 adjacent heads = a 128-row slice of the qkv projections).
Each core computes the full attention for its 2 heads over all 4096 rows and
a partial output projection; the host sums the 4 per-batch partials and adds
the (folded) biases.

On-chip layout strategy: everything is computed with the contraction dim on
partitions so no transposes are ever needed:
  Q^T,K^T [128d, 4096]  <- lhsT=W^T tiles, rhs=x^T
  V       [4096, 128d]  (natural; lhsT=x^T tile, rhs=Wv^T) + fused ones column
  S^T = K^T-stationary matmul, 2 heads row-packed (K=64 each) in the PE array
  P^T = exp(S^T) straight out of PSUM (no max-subtraction: |S|<~3).
  exp is split across THREE engines: most tiles on ScalarE
  (activation Exp with scale=4; S is pre-scaled by 1/4 in the weights, an
  exact power-of-2 fold), the rest via a custom DVE op computing a Horner
  deg-4 polynomial q~exp(u) on u=S/4 (1 pass) followed by two squaring
  passes on the otherwise-idle GPSIMD engine: P = ((q)^2)^2 = exp(S).
  O^T accum = (V|1)-stationary matmul over P^T; row 64 = softmax denominator
  normalize via reciprocal_approx_fast + PE outer-product broadcast; partial
  y^T = Wout^T slice-stationary matmul.
All biases are folded on the host.
"""

import os
import numpy as np
import ml_dtypes

B, N, D, H, HD = 2, 4096, 512, 8, 64
NCORES = 8
KT_TILES = 4      # D / 128 contraction tiles
JT = 32           # N / 128 key tiles
ICH = 8           # N / 512 query chunks
P = 128

MODE = os.environ.get("ATTN_KERNEL_MODE", "bf16")

# exp-poly coefficients: p(u) = 1 + u + C2 u^2 + C3 u^3 + C4 u^4 ~ exp(u)
# on [-0.8, 0.8]; P = p^4 ~ exp(4u), max rel err 1.3e-3.
EC2, EC3, EC4 = 0.50133404, 0.17126203, 0.03980059

# which jt positions (per ic chunk) take the DVE+GPSIMD exp route.
# Chosen clear of the norm-quanta DVE work (jts 2-7) and so every DVE PV
# (jt + DVE_LAG) lands within the same ic -> the ic's LAST issued PV stays
# jt31's (scalar, next-ic jt0) and finalize timing matches the o-pool
# rotation.  The last ic routes only the early ones to shorten the drain.
DVE_JTS = frozenset((8, 12, 16, 20, 24, 28))
DVE_JTS_LAST = frozenset((8, 12, 16, 20))
DVE_LAG = 4       # iterations between S-matmul and PV h0 issue (h1 lags +1)
SC_LAG = 1        # same for ScalarE-route tiles (slack against engine jitter)

_BUILD_CACHE = {}
_EXP_OP = None


def _register_exp_op():
    """Register the Horner deg-4 custom DVE op (idempotent)."""
    global _EXP_OP
    if _EXP_OP is not None:
        return _EXP_OP
    from concourse import dve_ops
    from concourse.dve_spec import Spec, Src0, C0, C1, C2, One, lower, _has_src1
    from concourse.dve_uop import DveOpSpec

    name = "EXP_P4_ANT"
    if name in dve_ops._SUB_OPCODE_FOR_NAME:
        _EXP_OP = next(o for o in dve_ops.OPS if o.name == name)
        return _EXP_OP
    body = (((C2 * Src0 + C1) * Src0 + C0) * Src0 + One) * Src0 + One
    spec = Spec(
        body=body,
        reference=lambda in0, in1, s0, s1, imm2: (
            (((imm2 * in0 + s1) * in0 + s0) * in0 + 1.0) * in0 + 1.0
        ).astype(np.float32),
    )
    row = max(dve_ops._SUB_OPCODE_FOR_NAME.values()) + 1
    dve_ops._SUB_OPCODE_FOR_NAME[name] = row
    tmp = DveOpSpec(name=name, opcode=row, uops=lower(spec, ver="v3"),
                    rd1_en=_has_src1(spec))
    op = dve_ops.DveOp(name, spec, subdim=False, uops_sha={"v3": tmp.sha("v3")})
    dve_ops.OPS.append(op)
    dve_ops.CUSTOM_DVE_SPECS[name] = spec
    _EXP_OP = op
    return op


def _np_dt(dt):
    import concourse.mybir as mybir
    return np.dtype(ml_dtypes.bfloat16) if dt == mybir.dt.bfloat16 else np.dtype(np.float32)


def _build(mode):
    """Build (and cache) the compiled Bass program for all cores (SPMD)."""
    if mode in _BUILD_CACHE:
        return _BUILD_CACHE[mode]

    import concourse.bacc as bacc
    import concourse.mybir as mybir
    import concourse.tile as tile
    from concourse.bass import _add_dep_helper
    from contextlib import ExitStack

    exp_op = _register_exp_op()

    f32 = mybir.dt.float32
    bf16 = mybir.dt.bfloat16
    if mode == "bf16":
        dt_qk, dt_pv = bf16, bf16
    elif mode == "mixed":
        dt_qk, dt_pv = f32, bf16
    else:
        dt_qk, dt_pv = f32, f32

    Exp = mybir.ActivationFunctionType.Exp
    mult = mybir.AluOpType.mult

    nc = bacc.Bacc(None, target_bir_lowering=False)
    xt_d = nc.dram_tensor("xt", [KT_TILES, P, N], dt_qk, kind="ExternalInput")
    wqt_d = nc.dram_tensor("wqt", [KT_TILES, P, P], dt_qk, kind="ExternalInput")
    wkt_d = nc.dram_tensor("wkt", [KT_TILES, P, P], dt_qk, kind="ExternalInput")
    wvt_d = nc.dram_tensor("wvt", [KT_TILES, P, P], dt_qk, kind="ExternalInput")
    wot_d = nc.dram_tensor("wot", [2, HD, D], dt_pv, kind="ExternalInput")
    bq_d = nc.dram_tensor("bq", [P, 1], f32, kind="ExternalInput")
    bk_d = nc.dram_tensor("bk", [P, 1], f32, kind="ExternalInput")
    yp_d = nc.dram_tensor("yp", [KT_TILES, P, N], f32, kind="ExternalOutput")

    def ics(i):
        return slice(i * 512, (i + 1) * 512)

    def jts(j):
        return slice(j * P, (j + 1) * P)

    def mts(m):
        return slice(m * P, (m + 1) * P)

    with tile.TileContext(nc) as tc, ExitStack() as ctx:
        const = ctx.enter_context(tc.tile_pool(name="const", bufs=1))
        sp = ctx.enter_context(tc.tile_pool(name="spool", bufs=2, space="PSUM"))
        op = ctx.enter_context(tc.tile_pool(name="opool", bufs=3, space="PSUM"))
        mp = ctx.enter_context(tc.tile_pool(name="mpool", bufs=1, space="PSUM"))
        pp = ctx.enter_context(tc.tile_pool(name="ppool", bufs=10))
        yep = ctx.enter_context(tc.tile_pool(name="yepool", bufs=3))
        rrp = ctx.enter_context(tc.tile_pool(name="rrpool", bufs=2))
        rbp = ctx.enter_context(tc.tile_pool(name="rbpool", bufs=2))
        drp = ctx.enter_context(tc.tile_pool(name="drpool", bufs=2))
        eyp = ctx.enter_context(tc.tile_pool(name="eypool", bufs=4))
        ezp = ctx.enter_context(tc.tile_pool(name="ezpool", bufs=4))

        xt = const.tile([P, KT_TILES, N], dt_qk, tag="xt")
        wqt = const.tile([P, KT_TILES, P], dt_qk, tag="wqt")
        wkt = const.tile([P, KT_TILES, P], dt_qk, tag="wkt")
        wvt = const.tile([P, KT_TILES, P], dt_qk, tag="wvt")
        # weight DMAs spread over three queues, K first (the kproj chains
        # gate the first S matmul), V/O last (not needed until jt0 of ic0)
        bq = const.tile([P, 1], f32, tag="bq")
        bk = const.tile([P, 1], f32, tag="bk")
        nc.scalar.dma_start(bq[:], bq_d[:])
        nc.scalar.dma_start(bk[:], bk_d[:])
        for k in range(KT_TILES):
            q = nc.gpsimd if k < 2 else nc.scalar
            q.dma_start(wkt[:, k, :], wkt_d[k])
        for k in range(KT_TILES):
            q = nc.gpsimd if k < 2 else nc.scalar
            q.dma_start(wqt[:, k, :], wqt_d[k])
        for k in range(KT_TILES):
            nc.gpsimd.dma_start(wvt[:, k, :], wvt_d[k])
        # x^T in column-major chunk order on ONE queue: the first column
        # blocks (all k-tiles) land at ~12% of the transfer, so the Q/K
        # projections and early attention start ~7us sooner than waiting for
        # whole k-tiles (total landing time is HBM-stack-BW-bound either way)
        XCH = 512
        for c in range(N // XCH):
            for k in range(KT_TILES):
                nc.sync.dma_start(xt[:, k, c * XCH:(c + 1) * XCH],
                                  xt_d[k][:, c * XCH:(c + 1) * XCH])
        wot = const.tile([HD, 2, D], dt_pv, tag="wot")
        for h in range(2):
            nc.scalar.dma_start(wot[:, h, :], wot_d[h])

        QT = const.tile([P, N], dt_qk, tag="QT")
        KT = const.tile([P, N], dt_qk, tag="KT")
        Vp = const.tile([P, JT, 130], dt_pv, tag="Vp")
        OT0 = const.tile([HD, N], dt_pv, tag="OT0")
        OT1 = const.tile([HD, N], dt_pv, tag="OT1")
        ones = const.tile([65, HD], dt_pv, tag="ones")
        nc.vector.memset(ones[64:65, :], 1.0)
        actwarm = const.tile([1, 1], f32, tag="actwarm")
        nc.vector.memset(actwarm[:], 0.0)
        nc.scalar.activation(actwarm[:], actwarm[:], Exp)
        # warm up gpsimd + DVE custom path before the steady loop
        gpw = const.tile([1, 8], f32, tag="gpw")
        nc.vector.memset(gpw[:], 1.0)
        nc.gpsimd.tensor_mul(gpw[:], gpw[:], gpw[:])
        dvw = const.tile([1, 8], f32, tag="dvw")
        nc.vector.memset(dvw[:], 0.0)
        nc.vector._custom_dve(exp_op, out=dvw[:], in0=dvw[:],
                              s0=EC2, s1=EC3, imm2=EC4)
        nc.vector.memset(Vp[:, :, 64:65], 1.0)
        nc.vector.memset(Vp[:, :, 129:130], 1.0)

        # ---- projection units (emitted interleaved into the attention loop
        # so the PE prefix before the first exp is tiny) ----
        def qproj_unit(ic):
            # Q^T[:, ic] (uses the otherwise-idle mp psum bank)
            ps = mp.tile([P, 512], f32, tag="mp", name=f"qp_{ic}")
            for k in range(KT_TILES):
                nc.tensor.matmul(ps[:], wqt[:, k, :], xt[:, k, ics(ic)],
                                 start=(k == 0), stop=(k == KT_TILES - 1))
            nc.vector.tensor_scalar_add(QT[:, ics(ic)], ps[:], bq[:, 0:1])

        def qproj_half_d(ic, half, after=None):
            # one [128,256] half of the deferred Q^T projection chain
            qs = slice(ics(ic).start + 256 * half, ics(ic).start + 256 * (half + 1))
            ps = mp.tile([P, 256], f32, tag="mp", name=f"qpd_{ic}_{half}")
            for k in range(KT_TILES):
                mm = nc.tensor.matmul(ps[:], wqt[:, k, :], xt[:, k, qs],
                                      start=(k == 0), stop=(k == KT_TILES - 1))
                if after is not None and k == 0:
                    _add_dep_helper(mm.ins, after.ins, sync=False,
                                    reason="defer qproj behind attention")
            nc.vector.tensor_scalar_add(QT[:, qs], ps[:], bq[:, 0:1])

        def kproj_unit(jc):
            # K^T[:, jc*512:(jc+1)*512]
            ps = mp.tile([P, 512], f32, tag="mp", name=f"kp_{jc}")
            for k in range(KT_TILES):
                nc.tensor.matmul(ps[:], wkt[:, k, :], xt[:, k, ics(jc)],
                                 start=(k == 0), stop=(k == KT_TILES - 1))
            nc.vector.tensor_scalar_add(KT[:, ics(jc)], ps[:], bk[:, 0:1])

        def vproj_unit(jt):
            # V[jt] (natural layout) + split into the two per-head Vp slabs
            ps = op.tile([P, P], f32, tag="o", name=f"vp_{jt}")
            for k in range(KT_TILES):
                nc.tensor.matmul(ps[:], xt[:, k, jts(jt)], wvt[:, k, :],
                                 start=(k == 0), stop=(k == KT_TILES - 1))
            nc.vector.tensor_copy(Vp[:, jt, 0:64], ps[:, 0:64])
            nc.vector.tensor_copy(Vp[:, jt, 65:129], ps[:, 64:128])

        def kproj_unit_s(jc):
            # K^T chunk on an s-pool slot (prefix only: runs parallel to the
            # qproj on the mp bank)
            ps = sp.tile([P, 512], f32, tag="s", name=f"kps_{jc}")
            for k in range(KT_TILES):
                nc.tensor.matmul(ps[:], wkt[:, k, :], xt[:, k, ics(jc)],
                                 start=(k == 0), stop=(k == KT_TILES - 1))
            nc.vector.tensor_scalar_add(KT[:, ics(jc)], ps[:], bk[:, 0:1])

        # upfront: Q chunk 0 on mp, K chunks 0+1 on the two s-pool slots
        qproj_unit(0)
        kproj_unit_s(0)
        kproj_unit_s(1)

        # ---- attention (software-pipelined emission: S/exp of step t, PV of
        # ready pending steps; ScalarE tiles are PV-ready after 1 iteration,
        # DVE-route tiles after DVE_LAG (their P goes DVE poly -> gpsimd
        # square -> gpsimd square).  finalize is staged as in the baseline:
        # DVE-only work right after the last PV; PE work several iterations
        # later so the PE FIFO never waits on the reciprocal. ----
        def finalize_a(ic, o0, o1):
            # PSUM evict + bf16 cast of the denominator rows (row 64); the
            # bf16 rows make the rb broadcast matmuls run at bf16 rate
            # (fp32 matmuls are ~5x more PE time).  In the drain tail the
            # h1 copies go to the otherwise-idle ScalarE so the four copies
            # run two-abreast.
            oss = []
            for i, o in enumerate((o0, o1)):
                os_ = rrp.tile([65, 512], f32, tag="os", name=f"os_{ic}_{i}")
                dr = drp.tile([65, 512], dt_pv, tag="dr", name=f"dr_{ic}_{i}")
                if in_tail and i == 1:
                    nc.scalar.copy(os_[:], o[:])
                    nc.scalar.copy(dr[64:65, :], o[64:65, :])
                else:
                    nc.vector.tensor_copy(os_[:], o[:])
                    nc.vector.tensor_copy(dr[64:65, :], o[64:65, :])
                oss.append((os_, dr))
            return oss

        def norm_quantum(ic, st, h, half, after=None):
            # normalize one head/half: OT[:, slice] = os[0:64] * (1/r) via PE
            # outer-product of the (bf16) denominator (no recip dep in PE
            # FIFO) then reciprocal+mul on DVE.
            os_, dr = st[h]
            OTt = (OT0, OT1)[h]
            ls = slice(256 * half, 256 * (half + 1))
            hs = slice(ics(ic).start + 256 * half, ics(ic).start + 256 * (half + 1))
            pool_h = mp if h == 0 else op
            rb = pool_h.tile([HD, 256], f32, tag="mp" if h == 0 else "o",
                             name=f"rb_{ic}_{h}_{half}")
            mm = nc.tensor.matmul(rb[:], ones[64:65, :], dr[64:65, ls],
                                  start=True, stop=True, tile_position=(64, 0))
            if after is not None:
                _add_dep_helper(mm.ins, after.ins, sync=False,
                                reason="defer finalize rb behind attention")
            rbs = rbp.tile([HD, 256], f32, tag="rbs", name=f"rbs_{ic}_{h}_{half}")
            nc.vector.reciprocal_approx_fast(out=rbs[:], in_=rb[:])
            nc.vector.tensor_mul(OTt[:, hs], os_[0:64, ls], rbs[:])

        def yproj_quantum(ic, mt, idx, after=None, tail=False):
            # one [128,512] slice of the partial output projection; tail
            # quanta alternate between the mp and (now idle) o-pool banks
            hs = ics(ic)
            pool_y = op if (tail and mt % 2 == 1) else mp
            yps = pool_y.tile([P, 512], f32,
                              tag="o" if (tail and mt % 2 == 1) else "mp",
                              name=f"yp_{ic}_{mt}")
            mm = nc.tensor.matmul(yps[:], wot[:, 0, mts(mt)], OT0[:, hs],
                                  start=True, stop=False)
            if after is not None:
                _add_dep_helper(mm.ins, after.ins, sync=False,
                                reason="defer finalize yproj behind attention")
            nc.tensor.matmul(yps[:], wot[:, 1, mts(mt)], OT1[:, hs],
                             start=False, stop=True)
            ye = yep.tile([P, 512], f32, tag="ye", name=f"ye_{ic}_{mt}")
            if tail:
                nc.scalar.copy(ye[:], yps[:])   # ScalarE is idle in the tail
            else:
                nc.vector.tensor_copy(ye[:], yps[:])
            nc.sync.dma_start(yp_d[mt, :, hs], ye[:])

        # quantum schedule within the NEXT chunk: (jt, fn(args))
        NORM_SCHED = [(2, (0, 0)), (4, (1, 0)), (5, (0, 1)), (7, (1, 1))]
        YP_SCHED = {9: 0, 11: 1, 13: 2, 15: 3}

        def emit_exp(s, ic, jt):
            """Emit the exp of s -> P tile; returns (p_tile, ready_lag).

            DVE route runs per 512-col half (one head each) so the
            DVE->gpsimd->gpsimd chain latency per consumed half is short."""
            p = pp.tile([P, 1024], dt_pv, tag="p")
            route = DVE_JTS_LAST if ic == ICH - 1 else DVE_JTS
            if jt in route:
                # per-half chains: gpsimd starts squaring h0 while the DVE
                # polys h1, so P-h0 lands well before its lag-4 PV deadline
                for h in range(2):
                    cs = slice(512 * h, 512 * (h + 1))
                    ey = eyp.tile([P, 512], f32, tag="ey", name=f"ey_{ic}_{jt}_{h}")
                    nc.vector._custom_dve(exp_op, out=ey[:], in0=s[:, cs],
                                          s0=EC2, s1=EC3, imm2=EC4)
                    ez = ezp.tile([P, 512], f32, tag="ez", name=f"ez_{ic}_{jt}_{h}")
                    nc.gpsimd.tensor_mul(ez[:], ey[:], ey[:])
                    nc.gpsimd.tensor_tensor(p[:, cs], ez[:], ez[:], mult)
                return p, DVE_LAG, True
            nc.scalar.activation(p[:], s[:], Exp, scale=4.0)
            return p, SC_LAG, False

        otiles = {}
        in_tail = False
        issued0 = {}         # ic -> PV h0 matmuls issued
        issued1 = {}         # ic -> PV h1 matmuls issued
        pend = []            # list of [p_tile, ic, jt, ready_g]
        pend_h1 = []         # deferred first-of-ic h1 halves (o-bank WAR slack)
        pend_b = None        # (ic, stage-a state) awaiting finalize_b
        g = 0

        def issue_pv_h1(ptile, pic, pjt):
            nonlocal pend_b
            o0, o1 = otiles[pic]
            nc.tensor.matmul(o1[:], Vp[:, pjt, 65:130], ptile[:, 512:1024],
                             start=(issued1[pic] == 0),
                             stop=(issued1[pic] == JT - 1))
            issued1[pic] += 1
            if issued0[pic] == JT and issued1[pic] == JT:
                pend_b = (pic, finalize_a(pic, o0, o1))

        def issue_pv(ptile, pic, pjt, defer_h1=False):
            # h0 immediately; h1 deferred one iteration for the ic's FIRST
            # pair (o-bank WAR slack vs the previous ic's o0 eviction) and
            # for DVE-route tiles (their h1 half lands one iteration later)
            o0, o1 = otiles[pic]
            nc.tensor.matmul(o0[:], Vp[:, pjt, 0:65], ptile[:, 0:512],
                             start=(issued0[pic] == 0),
                             stop=(issued0[pic] == JT - 1))
            issued0[pic] += 1
            if defer_h1 or (issued1[pic] == 0 and issued0[pic] == 1):
                pend_h1.append((ptile, pic, pjt))
            else:
                issue_pv_h1(ptile, pic, pjt)

        for ic in range(ICH):
            otiles[ic] = (op.tile([65, 512], f32, tag="o", name=f"o0_{ic}"),
                          op.tile([65, 512], f32, tag="o", name=f"o1_{ic}"))
            issued0[ic] = 0
            issued1[ic] = 0
            for jt in range(JT):
                s = sp.tile([P, 1024], f32, tag="s")
                last_s = nc.tensor.matmul(s[:, 0:512], KT[0:64, jts(jt)],
                                          QT[0:64, ics(ic)],
                                          start=True, stop=True,
                                          tile_position=(0, 0))
                nc.tensor.matmul(s[:, 512:1024], KT[64:128, jts(jt)],
                                 QT[64:128, ics(ic)],
                                 start=True, stop=True, tile_position=(64, 0))
                p, lag, isdve = emit_exp(s, ic, jt)
                pend.append([p, ic, jt, g + lag, isdve])
                # deferred h1 halves first, then all PVs ready at this point
                # (issue order is the PSUM accumulation order)
                for ent in pend_h1[:]:
                    pend_h1.remove(ent)
                    issue_pv_h1(*ent)
                for ent in [e for e in pend if e[3] <= g]:
                    pend.remove(ent)
                    issue_pv(ent[0], ent[1], ent[2], defer_h1=ent[4])
                # deferred proj + finalize stages, spread across the loop
                if ic == 0:
                    if jt == 0:
                        vproj_unit(0)
                        vproj_unit(1)
                    elif jt <= JT - 2:
                        vproj_unit(jt + 1)
                    if jt < 24 and jt % 4 == 0:
                        kproj_unit(2 + jt // 4)
                if pend_b is not None:
                    bic, st = pend_b
                    for sjt, (h, half) in NORM_SCHED:
                        if jt == sjt:
                            norm_quantum(bic, st, h, half, after=last_s)
                    if jt in YP_SCHED:
                        mt = YP_SCHED[jt]
                        yproj_quantum(bic, mt, mt, after=last_s)
                        if mt == 3:
                            pend_b = None
                if jt in (19, 21) and ic + 1 < ICH:
                    qproj_half_d(ic + 1, (jt - 19) // 2, after=last_s)
                g += 1
        # drain the pipeline tail
        in_tail = True
        for ent in pend_h1[:]:
            pend_h1.remove(ent)
            issue_pv_h1(*ent)
        for ent in sorted(pend, key=lambda e: e[3]):
            issue_pv(ent[0], ent[1], ent[2])
        pic, st = pend_b
        norm_quantum(pic, st, 0, 0)
        norm_quantum(pic, st, 1, 0)

        def yproj_tail_half(mt, half):
            # [128,256] half-quantum: starts as soon as that half of OT0/OT1
            # is normalized, overlapping the remaining norm work on DVE
            hs = slice(ics(pic).start + 256 * half,
                       ics(pic).start + 256 * (half + 1))
            pool_y = op if mt % 2 == 1 else mp
            yps = pool_y.tile([P, 256], f32,
                              tag="o" if mt % 2 == 1 else "mp",
                              name=f"ypt_{mt}_{half}")
            nc.tensor.matmul(yps[:], wot[:, 0, mts(mt)], OT0[:, hs],
                             start=True, stop=False)
            nc.tensor.matmul(yps[:], wot[:, 1, mts(mt)], OT1[:, hs],
                             start=False, stop=True)
            ye = yep.tile([P, 256], f32, tag="ye", name=f"yet_{mt}_{half}")
            nc.scalar.copy(ye[:], yps[:])
            nc.sync.dma_start(yp_d[mt, :, hs], ye[:])

        yproj_tail_half(0, 0)
        norm_quantum(pic, st, 0, 1)
        yproj_tail_half(1, 0)
        norm_quantum(pic, st, 1, 1)
        yproj_tail_half(2, 0)
        yproj_tail_half(3, 0)
        for mt in range(4):
            yproj_tail_half(mt, 1)

    nc.compile()
    _BUILD_CACHE[mode] = nc
    return nc


def _prep_inputs(x, qkv_w, qkv_b, out_w, mode):
    """Per-core input maps. Core c: batch c//4, head-pair c%4."""
    if mode == "bf16":
        dt_qk = np.dtype(ml_dtypes.bfloat16)
        dt_pv = dt_qk
    elif mode == "mixed":
        dt_qk = np.dtype(np.float32)
        dt_pv = np.dtype(ml_dtypes.bfloat16)
    else:
        dt_qk = np.dtype(np.float32)
        dt_pv = dt_qk

    x = np.asarray(x, np.float32)
    qkv_w = np.asarray(qkv_w, np.float32)
    qkv_b = np.asarray(qkv_b, np.float32)
    out_w = np.asarray(out_w, np.float32)

    xts = []
    for b in range(B):
        xt = np.ascontiguousarray(x[b].T).reshape(KT_TILES, P, N)
        xts.append(xt.astype(dt_qk))

    # q scale: 1/sqrt(hd)=0.125 plus an extra exact 1/4 so PSUM holds S/4
    # (ScalarE exp uses scale=4; the DVE poly route consumes S/4 directly)
    qs = 0.125 * 0.25
    in_maps = []
    for c in range(NCORES):
        b, m = divmod(c, 4)
        rs = slice(P * m, P * (m + 1))
        wq = (qs * qkv_w[0:D][rs]).T.reshape(KT_TILES, P, P)
        wk = qkv_w[D:2 * D][rs].T.reshape(KT_TILES, P, P)
        wv = qkv_w[2 * D:3 * D][rs].T.reshape(KT_TILES, P, P)
        wo = np.ascontiguousarray(out_w[:, rs].T).reshape(2, HD, D)
        in_maps.append({
            "xt": xts[b],
            "wqt": np.ascontiguousarray(wq).astype(dt_qk),
            "wkt": np.ascontiguousarray(wk).astype(dt_qk),
            "wvt": np.ascontiguousarray(wv).astype(dt_qk),
            "wot": wo.astype(dt_pv),
            "bq": (qs * qkv_b[0:D][rs]).reshape(P, 1).astype(np.float32),
            "bk": qkv_b[D:2 * D][rs].reshape(P, 1).astype(np.float32),
        })
    return in_maps


def _gather(results, qkv_b, out_w, out_b):
    # y[b] = (sum over the batch's 4 cores of yp)^T + out_w @ bv + out_b
    bias_vec = out_w.astype(np.float32) @ np.asarray(qkv_b, np.float32)[2 * D:3 * D] \
        + np.asarray(out_b, np.float32)
    y = np.empty((B, N, D), np.float32)
    for b in range(B):
        acc = np.zeros((D, N), np.float32)
        for m in range(4):
            acc += results[4 * b + m]["yp"].reshape(D, N)
        y[b] = acc.T + bias_vec
    return y


def _run(inputs, trace=False, tmpdir=None):
    from concourse.bass_utils import run_bass_kernel_spmd

    nc = _build(MODE)
    in_maps = _prep_inputs(inputs["x"], inputs["qkv_w"], inputs["qkv_b"],
                           inputs["out_w"], MODE)
    kw = {}
    if trace:
        kw = dict(trace=True, tmpdir=tmpdir)
    res = run_bass_kernel_spmd(nc, in_maps, core_ids=list(range(NCORES)), **kw)
    y = _gather(res.results, inputs["qkv_b"], inputs["out_w"], inputs["out_b"])
    return y, res


def kernel(x, qkv_w, qkv_b, out_w, out_b):
    y, _ = _run(dict(x=x, qkv_w=qkv_w, qkv_b=qkv_b, out_w=out_w, out_b=out_b))
    return y


# revision 42
# speedup vs baseline: 1.0084x; 1.0068x over previous
"""Multi-head self-attention Trainium2 kernel (8 NeuronCores, SPMD).

Problem: B=2, N=4096, D=512, H=8 heads of dim 64.
  qkv = x @ qkv_w.T + qkv_b ; per-head attention with softmax(QK^T/8) ;
  out = attn @ out_w.T + out_b

Sharding: 16 (batch, head) pairs -> 8 cores, each core owns one batch b and
one head-PAIR (2 adjacent heads = a 128-row slice of the qkv projections).
Each core computes the full attention for its 2 heads over all 4096 rows and
a partial output projection; the host sums the 4 per-batch partials and adds
the (folded) biases.

On-chip layout strategy: everything is computed with the contraction dim on
partitions so no transposes are ever needed:
  Q^T,K^T [128d, 4096]  <- lhsT=W^T tiles, rhs=x^T
  V       [4096, 128d]  (natural; lhsT=x^T tile, rhs=Wv^T) + fused ones column
  S^T = K^T-stationary matmul, 2 heads row-packed (K=64 each) in the PE array
  P^T = exp(S^T) straight out of PSUM (no max-subtraction: |S|<~3).
  exp is split across THREE engines: most tiles on ScalarE
  (activation Exp with scale=4; S is pre-scaled by 1/4 in the weights, an
  exact power-of-2 fold), the rest via a custom DVE op computing a Horner
  deg-4 polynomial q~exp(u) on u=S/4 (1 pass) followed by two squaring
  passes on the otherwise-idle GPSIMD engine: P = ((q)^2)^2 = exp(S).
  O^T accum = (V|1)-stationary matmul over P^T; row 64 = softmax denominator
  normalize via reciprocal_approx_fast + PE outer-product broadcast; partial
  y^T = Wout^T slice-stationary matmul.
All biases are folded on the host.
"""

import os
import numpy as np
import ml_dtypes

B, N, D, H, HD = 2, 4096, 512, 8, 64
NCORES = 8
KT_TILES = 4      # D / 128 contraction tiles
JT = 32           # N / 128 key tiles
ICH = 8           # N / 512 query chunks
P = 128

MODE = os.environ.get("ATTN_KERNEL_MODE", "bf16")

# exp-poly coefficients: p(u) = 1 + u + C2 u^2 + C3 u^3 + C4 u^4 ~ exp(u)
# on [-0.8, 0.8]; P = p^4 ~ exp(4u), max rel err 1.3e-3.
EC2, EC3, EC4 = 0.50133404, 0.17126203, 0.03980059

# which jt positions (per ic chunk) take the DVE+GPSIMD exp route.
# Chosen clear of the norm-quanta DVE work (jts 2-7) and so every DVE PV
# (jt + DVE_LAG) lands within the same ic -> the ic's LAST issued PV stays
# jt31's (scalar, next-ic jt0) and finalize timing matches the o-pool
# rotation.  The last ic routes only the early ones to shorten the drain.
DVE_JTS = frozenset((8, 12, 16, 20, 24, 28))
DVE_JTS_LAST = frozenset((8, 12, 16, 20))
DVE_LAG = 4       # iterations between S-matmul and PV h0 issue (h1 lags +1)
SC_LAG = 1        # same for ScalarE-route tiles (slack against engine jitter)

_BUILD_CACHE = {}
_EXP_OP = None


def _register_exp_op():
    """Register the Horner deg-4 custom DVE op (idempotent)."""
    global _EXP_OP
    if _EXP_OP is not None:
        return _EXP_OP
    from concourse import dve_ops
    from concourse.dve_spec import Spec, Src0, C0, C1, C2, One, lower, _has_src1
    from concourse.dve_uop import DveOpSpec

    name = "EXP_P4_ANT"
    if name in dve_ops._SUB_OPCODE_FOR_NAME:
        _EXP_OP = next(o for o in dve_ops.OPS if o.name == name)
        return _EXP_OP
    body = (((C2 * Src0 + C1) * Src0 + C0) * Src0 + One) * Src0 + One
    spec = Spec(
        body=body,
        reference=lambda in0, in1, s0, s1, imm2: (
            (((imm2 * in0 + s1) * in0 + s0) * in0 + 1.0) * in0 + 1.0
        ).astype(np.float32),
    )
    row = max(dve_ops._SUB_OPCODE_FOR_NAME.values()) + 1
    dve_ops._SUB_OPCODE_FOR_NAME[name] = row
    tmp = DveOpSpec(name=name, opcode=row, uops=lower(spec, ver="v3"),
                    rd1_en=_has_src1(spec))
    op = dve_ops.DveOp(name, spec, subdim=False, uops_sha={"v3": tmp.sha("v3")})
    dve_ops.OPS.append(op)
    dve_ops.CUSTOM_DVE_SPECS[name] = spec
    _EXP_OP = op
    return op


def _np_dt(dt):
    import concourse.mybir as mybir
    return np.dtype(ml_dtypes.bfloat16) if dt == mybir.dt.bfloat16 else np.dtype(np.float32)


def _build(mode):
    """Build (and cache) the compiled Bass program for all cores (SPMD)."""
    if mode in _BUILD_CACHE:
        return _BUILD_CACHE[mode]

    import concourse.bacc as bacc
    import concourse.mybir as mybir
    import concourse.tile as tile
    from concourse.bass import _add_dep_helper
    from contextlib import ExitStack

    exp_op = _register_exp_op()

    f32 = mybir.dt.float32
    bf16 = mybir.dt.bfloat16
    if mode == "bf16":
        dt_qk, dt_pv = bf16, bf16
    elif mode == "mixed":
        dt_qk, dt_pv = f32, bf16
    else:
        dt_qk, dt_pv = f32, f32

    Exp = mybir.ActivationFunctionType.Exp
    mult = mybir.AluOpType.mult

    nc = bacc.Bacc(None, target_bir_lowering=False)
    xt_d = nc.dram_tensor("xt", [KT_TILES, P, N], dt_qk, kind="ExternalInput")
    wqt_d = nc.dram_tensor("wqt", [KT_TILES, P, P], dt_qk, kind="ExternalInput")
    wkt_d = nc.dram_tensor("wkt", [KT_TILES, P, P], dt_qk, kind="ExternalInput")
    wvt_d = nc.dram_tensor("wvt", [KT_TILES, P, P], dt_qk, kind="ExternalInput")
    wot_d = nc.dram_tensor("wot", [2, HD, D], dt_pv, kind="ExternalInput")
    bq_d = nc.dram_tensor("bq", [P, 1], f32, kind="ExternalInput")
    bk_d = nc.dram_tensor("bk", [P, 1], f32, kind="ExternalInput")
    yp_d = nc.dram_tensor("yp", [KT_TILES, P, N], f32, kind="ExternalOutput")

    def ics(i):
        return slice(i * 512, (i + 1) * 512)

    def jts(j):
        return slice(j * P, (j + 1) * P)

    def mts(m):
        return slice(m * P, (m + 1) * P)

    with tile.TileContext(nc) as tc, ExitStack() as ctx:
        const = ctx.enter_context(tc.tile_pool(name="const", bufs=1))
        sp = ctx.enter_context(tc.tile_pool(name="spool", bufs=2, space="PSUM"))
        op = ctx.enter_context(tc.tile_pool(name="opool", bufs=3, space="PSUM"))
        mp = ctx.enter_context(tc.tile_pool(name="mpool", bufs=1, space="PSUM"))
        pp = ctx.enter_context(tc.tile_pool(name="ppool", bufs=10))
        yep = ctx.enter_context(tc.tile_pool(name="yepool", bufs=3))
        rrp = ctx.enter_context(tc.tile_pool(name="rrpool", bufs=2))
        rbp = ctx.enter_context(tc.tile_pool(name="rbpool", bufs=2))
        drp = ctx.enter_context(tc.tile_pool(name="drpool", bufs=2))
        eyp = ctx.enter_context(tc.tile_pool(name="eypool", bufs=4))
        ezp = ctx.enter_context(tc.tile_pool(name="ezpool", bufs=4))

        xt = const.tile([P, KT_TILES, N], dt_qk, tag="xt")
        wqt = const.tile([P, KT_TILES, P], dt_qk, tag="wqt")
        wkt = const.tile([P, KT_TILES, P], dt_qk, tag="wkt")
        wvt = const.tile([P, KT_TILES, P], dt_qk, tag="wvt")
        # weight DMAs spread over three queues, K first (the kproj chains
        # gate the first S matmul), V/O last (not needed until jt0 of ic0)
        bq = const.tile([P, 1], f32, tag="bq")
        bk = const.tile([P, 1], f32, tag="bk")
        nc.scalar.dma_start(bq[:], bq_d[:])
        nc.scalar.dma_start(bk[:], bk_d[:])
        for k in range(KT_TILES):
            q = nc.gpsimd if k < 2 else nc.scalar
            q.dma_start(wkt[:, k, :], wkt_d[k])
        for k in range(KT_TILES):
            q = nc.gpsimd if k < 2 else nc.scalar
            q.dma_start(wqt[:, k, :], wqt_d[k])
        for k in range(KT_TILES):
            nc.gpsimd.dma_start(wvt[:, k, :], wvt_d[k])
        # x^T in column-major chunk order on ONE queue: the first column
        # blocks (all k-tiles) land at ~12% of the transfer, so the Q/K
        # projections and early attention start ~7us sooner than waiting for
        # whole k-tiles (total landing time is HBM-stack-BW-bound either way)
        XCH = 512
        for c in range(N // XCH):
            for k in range(KT_TILES):
                nc.sync.dma_start(xt[:, k, c * XCH:(c + 1) * XCH],
                                  xt_d[k][:, c * XCH:(c + 1) * XCH])
        wot = const.tile([HD, 2, D], dt_pv, tag="wot")
        for h in range(2):
            nc.scalar.dma_start(wot[:, h, :], wot_d[h])

        QT = const.tile([P, N], dt_qk, tag="QT")
        KT = const.tile([P, N], dt_qk, tag="KT")
        Vp = const.tile([P, JT, 130], dt_pv, tag="Vp")
        OT0 = const.tile([HD, N], dt_pv, tag="OT0")
        OT1 = const.tile([HD, N], dt_pv, tag="OT1")
        ones = const.tile([65, HD], dt_pv, tag="ones")
        nc.vector.memset(ones[64:65, :], 1.0)
        actwarm = const.tile([1, 1], f32, tag="actwarm")
        nc.vector.memset(actwarm[:], 0.0)
        nc.scalar.activation(actwarm[:], actwarm[:], Exp)
        # warm up gpsimd + DVE custom path before the steady loop
        gpw = const.tile([1, 8], f32, tag="gpw")
        nc.vector.memset(gpw[:], 1.0)
        nc.gpsimd.tensor_mul(gpw[:], gpw[:], gpw[:])
        dvw = const.tile([1, 8], f32, tag="dvw")
        nc.vector.memset(dvw[:], 0.0)
        nc.vector._custom_dve(exp_op, out=dvw[:], in0=dvw[:],
                              s0=EC2, s1=EC3, imm2=EC4)
        nc.vector.memset(Vp[:, :, 64:65], 1.0)
        nc.vector.memset(Vp[:, :, 129:130], 1.0)

        # ---- projection units (emitted interleaved into the attention loop
        # so the PE prefix before the first exp is tiny) ----
        def qproj_unit(ic):
            # Q^T[:, ic] (uses the otherwise-idle mp psum bank)
            ps = mp.tile([P, 512], f32, tag="mp", name=f"qp_{ic}")
            for k in range(KT_TILES):
                nc.tensor.matmul(ps[:], wqt[:, k, :], xt[:, k, ics(ic)],
                                 start=(k == 0), stop=(k == KT_TILES - 1))
            nc.vector.tensor_scalar_add(QT[:, ics(ic)], ps[:], bq[:, 0:1])

        def qproj_half_d(ic, half, after=None):
            # one [128,256] half of the deferred Q^T projection chain
            qs = slice(ics(ic).start + 256 * half, ics(ic).start + 256 * (half + 1))
            ps = mp.tile([P, 256], f32, tag="mp", name=f"qpd_{ic}_{half}")
            for k in range(KT_TILES):
                mm = nc.tensor.matmul(ps[:], wqt[:, k, :], xt[:, k, qs],
                                      start=(k == 0), stop=(k == KT_TILES - 1))
                if after is not None and k == 0:
                    _add_dep_helper(mm.ins, after.ins, sync=False,
                                    reason="defer qproj behind attention")
            nc.vector.tensor_scalar_add(QT[:, qs], ps[:], bq[:, 0:1])

        def kproj_unit(jc):
            # K^T[:, jc*512:(jc+1)*512]
            ps = mp.tile([P, 512], f32, tag="mp", name=f"kp_{jc}")
            for k in range(KT_TILES):
                nc.tensor.matmul(ps[:], wkt[:, k, :], xt[:, k, ics(jc)],
                                 start=(k == 0), stop=(k == KT_TILES - 1))
            nc.vector.tensor_scalar_add(KT[:, ics(jc)], ps[:], bk[:, 0:1])

        def vproj_unit(jt):
            # V[jt] (natural layout) + split into the two per-head Vp slabs
            ps = op.tile([P, P], f32, tag="o", name=f"vp_{jt}")
            for k in range(KT_TILES):
                nc.tensor.matmul(ps[:], xt[:, k, jts(jt)], wvt[:, k, :],
                                 start=(k == 0), stop=(k == KT_TILES - 1))
            nc.vector.tensor_copy(Vp[:, jt, 0:64], ps[:, 0:64])
            nc.vector.tensor_copy(Vp[:, jt, 65:129], ps[:, 64:128])

        def kproj_unit_s(jc):
            # K^T chunk on an s-pool slot (prefix only: runs parallel to the
            # qproj on the mp bank)
            ps = sp.tile([P, 512], f32, tag="s", name=f"kps_{jc}")
            for k in range(KT_TILES):
                nc.tensor.matmul(ps[:], wkt[:, k, :], xt[:, k, ics(jc)],
                                 start=(k == 0), stop=(k == KT_TILES - 1))
            nc.vector.tensor_scalar_add(KT[:, ics(jc)], ps[:], bk[:, 0:1])

        # upfront: Q chunk 0 on mp, K chunks 0+1 on the two s-pool slots
        qproj_unit(0)
        kproj_unit_s(0)
        kproj_unit_s(1)

        # ---- attention (software-pipelined emission: S/exp of step t, PV of
        # ready pending steps; ScalarE tiles are PV-ready after 1 iteration,
        # DVE-route tiles after DVE_LAG (their P goes DVE poly -> gpsimd
        # square -> gpsimd square).  finalize is staged as in the baseline:
        # DVE-only work right after the last PV; PE work several iterations
        # later so the PE FIFO never waits on the reciprocal. ----
        def finalize_a(ic, o0, o1):
            # PSUM evict + bf16 cast of the denominator rows (row 64); the
            # bf16 rows make the rb broadcast matmuls run at bf16 rate
            # (fp32 matmuls are ~5x more PE time).  In the drain tail the
            # h1 copies go to the otherwise-idle ScalarE so the four copies
            # run two-abreast.
            oss = []
            for i, o in enumerate((o0, o1)):
                os_ = rrp.tile([65, 512], f32, tag="os", name=f"os_{ic}_{i}")
                dr = drp.tile([65, 512], dt_pv, tag="dr", name=f"dr_{ic}_{i}")
                if in_tail and i == 1:
                    nc.scalar.copy(os_[:], o[:])
                    nc.scalar.copy(dr[64:65, :], o[64:65, :])
                else:
                    nc.vector.tensor_copy(os_[:], o[:])
                    nc.vector.tensor_copy(dr[64:65, :], o[64:65, :])
                oss.append((os_, dr))
            return oss

        def norm_quantum(ic, st, h, half, after=None):
            # normalize one head/half: OT[:, slice] = os[0:64] * (1/r) via PE
            # outer-product of the (bf16) denominator (no recip dep in PE
            # FIFO) then reciprocal+mul on DVE.
            os_, dr = st[h]
            OTt = (OT0, OT1)[h]
            ls = slice(256 * half, 256 * (half + 1))
            hs = slice(ics(ic).start + 256 * half, ics(ic).start + 256 * (half + 1))
            pool_h = mp if h == 0 else op
            rb = pool_h.tile([HD, 256], f32, tag="mp" if h == 0 else "o",
                             name=f"rb_{ic}_{h}_{half}")
            mm = nc.tensor.matmul(rb[:], ones[64:65, :], dr[64:65, ls],
                                  start=True, stop=True, tile_position=(64, 0))
            if after is not None:
                _add_dep_helper(mm.ins, after.ins, sync=False,
                                reason="defer finalize rb behind attention")
            rbs = rbp.tile([HD, 256], f32, tag="rbs", name=f"rbs_{ic}_{h}_{half}")
            nc.vector.reciprocal_approx_fast(out=rbs[:], in_=rb[:])
            nc.vector.tensor_mul(OTt[:, hs], os_[0:64, ls], rbs[:])

        def yproj_quantum(ic, mt, idx, after=None, tail=False):
            # one [128,512] slice of the partial output projection; tail
            # quanta alternate between the mp and (now idle) o-pool banks
            hs = ics(ic)
            pool_y = op if (tail and mt % 2 == 1) else mp
            yps = pool_y.tile([P, 512], f32,
                              tag="o" if (tail and mt % 2 == 1) else "mp",
                              name=f"yp_{ic}_{mt}")
            mm = nc.tensor.matmul(yps[:], wot[:, 0, mts(mt)], OT0[:, hs],
                                  start=True, stop=False)
            if after is not None:
                _add_dep_helper(mm.ins, after.ins, sync=False,
                                reason="defer finalize yproj behind attention")
            nc.tensor.matmul(yps[:], wot[:, 1, mts(mt)], OT1[:, hs],
                             start=False, stop=True)
            ye = yep.tile([P, 512], f32, tag="ye", name=f"ye_{ic}_{mt}")
            if tail:
                nc.scalar.copy(ye[:], yps[:])   # ScalarE is idle in the tail
            else:
                nc.vector.tensor_copy(ye[:], yps[:])
            nc.sync.dma_start(yp_d[mt, :, hs], ye[:])

        # quantum schedule within the NEXT chunk: (jt, fn(args))
        NORM_SCHED = [(2, (0, 0)), (4, (1, 0)), (5, (0, 1)), (7, (1, 1))]
        YP_SCHED = {9: 0, 11: 1, 13: 2, 15: 3}

        def emit_exp(s, ic, jt):
            """Emit the exp of s -> P tile; returns (p_tile, ready_lag).

            DVE route runs per 512-col half (one head each) so the
            DVE->gpsimd->gpsimd chain latency per consumed half is short."""
            p = pp.tile([P, 1024], dt_pv, tag="p")
            route = DVE_JTS_LAST if ic == ICH - 1 else DVE_JTS
            if jt in route:
                # per-half chains: gpsimd starts squaring h0 while the DVE
                # polys h1, so P-h0 lands well before its lag-4 PV deadline
                for h in range(2):
                    cs = slice(512 * h, 512 * (h + 1))
                    ey = eyp.tile([P, 512], f32, tag="ey", name=f"ey_{ic}_{jt}_{h}")
                    nc.vector._custom_dve(exp_op, out=ey[:], in0=s[:, cs],
                                          s0=EC2, s1=EC3, imm2=EC4)
                    ez = ezp.tile([P, 512], f32, tag="ez", name=f"ez_{ic}_{jt}_{h}")
                    nc.gpsimd.tensor_mul(ez[:], ey[:], ey[:])
                    nc.gpsimd.tensor_tensor(p[:, cs], ez[:], ez[:], mult)
                return p, DVE_LAG, True
            nc.scalar.activation(p[:], s[:], Exp, scale=4.0)
            return p, SC_LAG, False

        otiles = {}
        in_tail = False
        issued0 = {}         # ic -> PV h0 matmuls issued
        issued1 = {}         # ic -> PV h1 matmuls issued
        pend = []            # list of [p_tile, ic, jt, ready_g]
        pend_h1 = []         # deferred first-of-ic h1 halves (o-bank WAR slack)
        pend_b = None        # (ic, stage-a state) awaiting finalize_b
        g = 0

        def issue_pv_h1(ptile, pic, pjt):
            nonlocal pend_b
            o0, o1 = otiles[pic]
            nc.tensor.matmul(o1[:], Vp[:, pjt, 65:130], ptile[:, 512:1024],
                             start=(issued1[pic] == 0),
                             stop=(issued1[pic] == JT - 1))
            issued1[pic] += 1
            if issued0[pic] == JT and issued1[pic] == JT:
                pend_b = (pic, finalize_a(pic, o0, o1))

        def issue_pv(ptile, pic, pjt, defer_h1=False):
            # h0 immediately; h1 deferred one iteration for the ic's FIRST
            # pair (o-bank WAR slack vs the previous ic's o0 eviction) and
            # for DVE-route tiles (their h1 half lands one iteration later)
            o0, o1 = otiles[pic]
            nc.tensor.matmul(o0[:], Vp[:, pjt, 0:65], ptile[:, 0:512],
                             start=(issued0[pic] == 0),
                             stop=(issued0[pic] == JT - 1))
            issued0[pic] += 1
            if defer_h1 or (issued1[pic] == 0 and issued0[pic] == 1):
                pend_h1.append((ptile, pic, pjt))
            else:
                issue_pv_h1(ptile, pic, pjt)

        for ic in range(ICH):
            otiles[ic] = (op.tile([65, 512], f32, tag="o", name=f"o0_{ic}"),
                          op.tile([65, 512], f32, tag="o", name=f"o1_{ic}"))
            issued0[ic] = 0
            issued1[ic] = 0
            for jt in range(JT):
                s = sp.tile([P, 1024], f32, tag="s")
                last_s = nc.tensor.matmul(s[:, 0:512], KT[0:64, jts(jt)],
                                          QT[0:64, ics(ic)],
                                          start=True, stop=True,
                                          tile_position=(0, 0))
                nc.tensor.matmul(s[:, 512:1024], KT[64:128, jts(jt)],
                                 QT[64:128, ics(ic)],
                                 start=True, stop=True, tile_position=(64, 0))
                p, lag, isdve = emit_exp(s, ic, jt)
                pend.append([p, ic, jt, g + lag, isdve])
                # deferred h1 halves first, then all PVs ready at this point
                # (issue order is the PSUM accumulation order)
                for ent in pend_h1[:]:
                    pend_h1.remove(ent)
                    issue_pv_h1(*ent)
                for ent in [e for e in pend if e[3] <= g]:
                    pend.remove(ent)
                    issue_pv(ent[0], ent[1], ent[2], defer_h1=ent[4])
                # deferred proj + finalize stages, spread across the loop
                if ic == 0:
                    if jt == 0:
                        vproj_unit(0)
                        vproj_unit(1)
                    elif jt <= JT - 2:
                        vproj_unit(jt + 1)
                    if jt < 24 and jt % 4 == 0:
                        kproj_unit(2 + jt // 4)
                if pend_b is not None:
                    bic, st = pend_b
                    for sjt, (h, half) in NORM_SCHED:
                        if jt == sjt:
                            norm_quantum(bic, st, h, half, after=last_s)
                    if jt in YP_SCHED:
                        mt = YP_SCHED[jt]
                        yproj_quantum(bic, mt, mt, after=last_s)
                        if mt == 3:
                            pend_b = None
                if jt in (19, 21) and ic + 1 < ICH:
                    qproj_half_d(ic + 1, (jt - 19) // 2, after=last_s)
                g += 1
        # drain the pipeline tail
        in_tail = True
        for ent in pend_h1[:]:
            pend_h1.remove(ent)
            issue_pv_h1(*ent)
        for ent in sorted(pend, key=lambda e: e[3]):
            issue_pv(ent[0], ent[1], ent[2])
        pic, st = pend_b
        for _, (h, half) in NORM_SCHED:
            norm_quantum(pic, st, h, half)
        for mt in range(4):
            yproj_quantum(pic, mt, mt, tail=True)

    nc.compile()
    _BUILD_CACHE[mode] = nc
    return nc


def _prep_inputs(x, qkv_w, qkv_b, out_w, mode):
    """Per-core input maps. Core c: batch c//4, head-pair c%4."""
    if mode == "bf16":
        dt_qk = np.dtype(ml_dtypes.bfloat16)
        dt_pv = dt_qk
    elif mode == "mixed":
        dt_qk = np.dtype(np.float32)
        dt_pv = np.dtype(ml_dtypes.bfloat16)
    else:
        dt_qk = np.dtype(np.float32)
        dt_pv = dt_qk

    x = np.asarray(x, np.float32)
    qkv_w = np.asarray(qkv_w, np.float32)
    qkv_b = np.asarray(qkv_b, np.float32)
    out_w = np.asarray(out_w, np.float32)

    xts = []
    for b in range(B):
        xt = np.ascontiguousarray(x[b].T).reshape(KT_TILES, P, N)
        xts.append(xt.astype(dt_qk))

    # q scale: 1/sqrt(hd)=0.125 plus an extra exact 1/4 so PSUM holds S/4
    # (ScalarE exp uses scale=4; the DVE poly route consumes S/4 directly)
    qs = 0.125 * 0.25
    in_maps = []
    for c in range(NCORES):
        b, m = divmod(c, 4)
        rs = slice(P * m, P * (m + 1))
        wq = (qs * qkv_w[0:D][rs]).T.reshape(KT_TILES, P, P)
        wk = qkv_w[D:2 * D][rs].T.reshape(KT_TILES, P, P)
        wv = qkv_w[2 * D:3 * D][rs].T.reshape(KT_TILES, P, P)
        wo = np.ascontiguousarray(out_w[:, rs].T).reshape(2, HD, D)
        in_maps.append({
            "xt": xts[b],
            "wqt": np.ascontiguousarray(wq).astype(dt_qk),
            "wkt": np.ascontiguousarray(wk).astype(dt_qk),
            "wvt": np.ascontiguousarray(wv).astype(dt_qk),
            "wot": wo.astype(dt_pv),
            "bq": (qs * qkv_b[0:D][rs]).reshape(P, 1).astype(np.float32),
            "bk": qkv_b[D:2 * D][rs].reshape(P, 1).astype(np.float32),
        })
    return in_maps


def _gather(results, qkv_b, out_w, out_b):
    # y[b] = (sum over the batch's 4 cores of yp)^T + out_w @ bv + out_b
    bias_vec = out_w.astype(np.float32) @ np.asarray(qkv_b, np.float32)[2 * D:3 * D] \
        + np.asarray(out_b, np.float32)
    y = np.empty((B, N, D), np.float32)
    for b in range(B):
        acc = np.zeros((D, N), np.float32)
        for m in range(4):
            acc += results[4 * b + m]["yp"].reshape(D, N)
        y[b] = acc.T + bias_vec
    return y


def _run(inputs, trace=False, tmpdir=None):
    from concourse.bass_utils import run_bass_kernel_spmd

    nc = _build(MODE)
    in_maps = _prep_inputs(inputs["x"], inputs["qkv_w"], inputs["qkv_b"],
                           inputs["out_w"], MODE)
    kw = {}
    if trace:
        kw = dict(trace=True, tmpdir=tmpdir)
    res = run_bass_kernel_spmd(nc, in_maps, core_ids=list(range(NCORES)), **kw)
    y = _gather(res.results, inputs["qkv_b"], inputs["out_w"], inputs["out_b"])
    return y, res


def kernel(x, qkv_w, qkv_b, out_w, out_b):
    y, _ = _run(dict(x=x, qkv_w=qkv_w, qkv_b=qkv_b, out_w=out_w, out_b=out_b))
    return y


# revision 43
# speedup vs baseline: 1.0102x; 1.0018x over previous
"""Multi-head self-attention Trainium2 kernel (8 NeuronCores, SPMD).

Problem: B=2, N=4096, D=512, H=8 heads of dim 64.
  qkv = x @ qkv_w.T + qkv_b ; per-head attention with softmax(QK^T/8) ;
  out = attn @ out_w.T + out_b

Sharding: 16 (batch, head) pairs -> 8 cores, each core owns one batch b and
one head-PAIR (2 adjacent heads = a 128-row slice of the qkv projections).
Each core computes the full attention for its 2 heads over all 4096 rows and
a partial output projection; the host sums the 4 per-batch partials and adds
the (folded) biases.

On-chip layout strategy: everything is computed with the contraction dim on
partitions so no transposes are ever needed:
  Q^T,K^T [128d, 4096]  <- lhsT=W^T tiles, rhs=x^T
  V       [4096, 128d]  (natural; lhsT=x^T tile, rhs=Wv^T) + fused ones column
  S^T = K^T-stationary matmul, 2 heads row-packed (K=64 each) in the PE array
  P^T = exp(S^T) straight out of PSUM (no max-subtraction: |S|<~3).
  exp is split across THREE engines: most tiles on ScalarE
  (activation Exp with scale=4; S is pre-scaled by 1/4 in the weights, an
  exact power-of-2 fold), the rest via a custom DVE op computing a Horner
  deg-4 polynomial q~exp(u) on u=S/4 (1 pass) followed by two squaring
  passes on the otherwise-idle GPSIMD engine: P = ((q)^2)^2 = exp(S).
  O^T accum = (V|1)-stationary matmul over P^T; row 64 = softmax denominator
  normalize via reciprocal_approx_fast + PE outer-product broadcast; partial
  y^T = Wout^T slice-stationary matmul.
All biases are folded on the host.
"""

import os
import numpy as np
import ml_dtypes

B, N, D, H, HD = 2, 4096, 512, 8, 64
NCORES = 8
KT_TILES = 4      # D / 128 contraction tiles
JT = 32           # N / 128 key tiles
ICH = 8           # N / 512 query chunks
P = 128

MODE = os.environ.get("ATTN_KERNEL_MODE", "bf16")

# exp-poly coefficients: p(u) = 1 + u + C2 u^2 + C3 u^3 + C4 u^4 ~ exp(u)
# on [-0.8, 0.8]; P = p^4 ~ exp(4u), max rel err 1.3e-3.
EC2, EC3, EC4 = 0.50133404, 0.17126203, 0.03980059

# which jt positions (per ic chunk) take the DVE+GPSIMD exp route.
# Chosen clear of the norm-quanta DVE work (jts 2-7) and so every DVE PV
# (jt + DVE_LAG) lands within the same ic -> the ic's LAST issued PV stays
# jt31's (scalar, next-ic jt0) and finalize timing matches the o-pool
# rotation.  The last ic routes only the early ones to shorten the drain.
DVE_JTS = frozenset((8, 12, 16, 20, 24, 28))
DVE_JTS_LAST = frozenset((8, 12, 16, 20))
DVE_LAG = 4       # iterations between S-matmul and PV h0 issue (h1 lags +1)
SC_LAG = 1        # same for ScalarE-route tiles (slack against engine jitter)

_BUILD_CACHE = {}
_EXP_OP = None


def _register_exp_op():
    """Register the Horner deg-4 custom DVE op (idempotent)."""
    global _EXP_OP
    if _EXP_OP is not None:
        return _EXP_OP
    from concourse import dve_ops
    from concourse.dve_spec import Spec, Src0, C0, C1, C2, One, lower, _has_src1
    from concourse.dve_uop import DveOpSpec

    name = "EXP_P4_ANT"
    if name in dve_ops._SUB_OPCODE_FOR_NAME:
        _EXP_OP = next(o for o in dve_ops.OPS if o.name == name)
        return _EXP_OP
    body = (((C2 * Src0 + C1) * Src0 + C0) * Src0 + One) * Src0 + One
    spec = Spec(
        body=body,
        reference=lambda in0, in1, s0, s1, imm2: (
            (((imm2 * in0 + s1) * in0 + s0) * in0 + 1.0) * in0 + 1.0
        ).astype(np.float32),
    )
    row = max(dve_ops._SUB_OPCODE_FOR_NAME.values()) + 1
    dve_ops._SUB_OPCODE_FOR_NAME[name] = row
    tmp = DveOpSpec(name=name, opcode=row, uops=lower(spec, ver="v3"),
                    rd1_en=_has_src1(spec))
    op = dve_ops.DveOp(name, spec, subdim=False, uops_sha={"v3": tmp.sha("v3")})
    dve_ops.OPS.append(op)
    dve_ops.CUSTOM_DVE_SPECS[name] = spec
    _EXP_OP = op
    return op


def _np_dt(dt):
    import concourse.mybir as mybir
    return np.dtype(ml_dtypes.bfloat16) if dt == mybir.dt.bfloat16 else np.dtype(np.float32)


def _build(mode):
    """Build (and cache) the compiled Bass program for all cores (SPMD)."""
    if mode in _BUILD_CACHE:
        return _BUILD_CACHE[mode]

    import concourse.bacc as bacc
    import concourse.mybir as mybir
    import concourse.tile as tile
    from concourse.bass import _add_dep_helper
    from contextlib import ExitStack

    exp_op = _register_exp_op()

    f32 = mybir.dt.float32
    bf16 = mybir.dt.bfloat16
    if mode == "bf16":
        dt_qk, dt_pv = bf16, bf16
    elif mode == "mixed":
        dt_qk, dt_pv = f32, bf16
    else:
        dt_qk, dt_pv = f32, f32

    Exp = mybir.ActivationFunctionType.Exp
    mult = mybir.AluOpType.mult

    nc = bacc.Bacc(None, target_bir_lowering=False)
    xt_d = nc.dram_tensor("xt", [KT_TILES, P, N], dt_qk, kind="ExternalInput")
    wqt_d = nc.dram_tensor("wqt", [KT_TILES, P, P], dt_qk, kind="ExternalInput")
    wkt_d = nc.dram_tensor("wkt", [KT_TILES, P, P], dt_qk, kind="ExternalInput")
    wvt_d = nc.dram_tensor("wvt", [KT_TILES, P, P], dt_qk, kind="ExternalInput")
    wot_d = nc.dram_tensor("wot", [2, HD, D], dt_pv, kind="ExternalInput")
    bq_d = nc.dram_tensor("bq", [P, 1], f32, kind="ExternalInput")
    bk_d = nc.dram_tensor("bk", [P, 1], f32, kind="ExternalInput")
    yp_d = nc.dram_tensor("yp", [KT_TILES, P, N], f32, kind="ExternalOutput")

    def ics(i):
        return slice(i * 512, (i + 1) * 512)

    def jts(j):
        return slice(j * P, (j + 1) * P)

    def mts(m):
        return slice(m * P, (m + 1) * P)

    with tile.TileContext(nc) as tc, ExitStack() as ctx:
        const = ctx.enter_context(tc.tile_pool(name="const", bufs=1))
        sp = ctx.enter_context(tc.tile_pool(name="spool", bufs=2, space="PSUM"))
        op = ctx.enter_context(tc.tile_pool(name="opool", bufs=3, space="PSUM"))
        mp = ctx.enter_context(tc.tile_pool(name="mpool", bufs=1, space="PSUM"))
        pp = ctx.enter_context(tc.tile_pool(name="ppool", bufs=10))
        yep = ctx.enter_context(tc.tile_pool(name="yepool", bufs=3))
        rrp = ctx.enter_context(tc.tile_pool(name="rrpool", bufs=2))
        rbp = ctx.enter_context(tc.tile_pool(name="rbpool", bufs=2))
        drp = ctx.enter_context(tc.tile_pool(name="drpool", bufs=2))
        eyp = ctx.enter_context(tc.tile_pool(name="eypool", bufs=4))
        ezp = ctx.enter_context(tc.tile_pool(name="ezpool", bufs=4))

        xt = const.tile([P, KT_TILES, N], dt_qk, tag="xt")
        wqt = const.tile([P, KT_TILES, P], dt_qk, tag="wqt")
        wkt = const.tile([P, KT_TILES, P], dt_qk, tag="wkt")
        wvt = const.tile([P, KT_TILES, P], dt_qk, tag="wvt")
        # DMA priority plan: the prefix critical path is qproj (needs wqt +
        # x chunk 0) then kproj (wkt + x chunk 0) on the PE.  Spread chunk
        # 0's k-tiles over two queues so all of c0 lands by ~9us instead of
        # trickling in at the sync queue's one-transfer-per-600ns rate.
        bq = const.tile([P, 1], f32, tag="bq")
        bk = const.tile([P, 1], f32, tag="bk")
        XCH = 512
        for k in range(KT_TILES):
            nc.gpsimd.dma_start(wqt[:, k, :], wqt_d[k])
        nc.gpsimd.dma_start(wkt[:, 0, :], wkt_d[0])
        nc.gpsimd.dma_start(wkt[:, 1, :], wkt_d[1])
        for k in range(KT_TILES):
            nc.gpsimd.dma_start(wvt[:, k, :], wvt_d[k])
        nc.scalar.dma_start(bq[:], bq_d[:])
        nc.scalar.dma_start(bk[:], bk_d[:])
        nc.scalar.dma_start(xt[:, 2, 0:XCH], xt_d[2][:, 0:XCH])
        nc.scalar.dma_start(xt[:, 3, 0:XCH], xt_d[3][:, 0:XCH])
        nc.scalar.dma_start(wkt[:, 2, :], wkt_d[2])
        nc.scalar.dma_start(wkt[:, 3, :], wkt_d[3])
        nc.sync.dma_start(xt[:, 0, 0:XCH], xt_d[0][:, 0:XCH])
        nc.sync.dma_start(xt[:, 1, 0:XCH], xt_d[1][:, 0:XCH])
        for c in range(1, N // XCH):
            for k in range(KT_TILES):
                nc.sync.dma_start(xt[:, k, c * XCH:(c + 1) * XCH],
                                  xt_d[k][:, c * XCH:(c + 1) * XCH])
        wot = const.tile([HD, 2, D], dt_pv, tag="wot")
        for h in range(2):
            nc.scalar.dma_start(wot[:, h, :], wot_d[h])


        QT = const.tile([P, N], dt_qk, tag="QT")
        KT = const.tile([P, N], dt_qk, tag="KT")
        Vp = const.tile([P, JT, 130], dt_pv, tag="Vp")
        OT0 = const.tile([HD, N], dt_pv, tag="OT0")
        OT1 = const.tile([HD, N], dt_pv, tag="OT1")
        ones = const.tile([65, HD], dt_pv, tag="ones")
        nc.vector.memset(ones[64:65, :], 1.0)
        actwarm = const.tile([1, 1], f32, tag="actwarm")
        nc.vector.memset(actwarm[:], 0.0)
        nc.scalar.activation(actwarm[:], actwarm[:], Exp)
        # warm up gpsimd + DVE custom path before the steady loop
        gpw = const.tile([1, 8], f32, tag="gpw")
        nc.vector.memset(gpw[:], 1.0)
        nc.gpsimd.tensor_mul(gpw[:], gpw[:], gpw[:])
        dvw = const.tile([1, 8], f32, tag="dvw")
        nc.vector.memset(dvw[:], 0.0)
        nc.vector._custom_dve(exp_op, out=dvw[:], in0=dvw[:],
                              s0=EC2, s1=EC3, imm2=EC4)
        nc.vector.memset(Vp[:, :, 64:65], 1.0)
        nc.vector.memset(Vp[:, :, 129:130], 1.0)

        # ---- projection units (emitted interleaved into the attention loop
        # so the PE prefix before the first exp is tiny) ----
        def qproj_unit(ic):
            # Q^T[:, ic] (uses the otherwise-idle mp psum bank)
            ps = mp.tile([P, 512], f32, tag="mp", name=f"qp_{ic}")
            for k in range(KT_TILES):
                nc.tensor.matmul(ps[:], wqt[:, k, :], xt[:, k, ics(ic)],
                                 start=(k == 0), stop=(k == KT_TILES - 1))
            nc.vector.tensor_scalar_add(QT[:, ics(ic)], ps[:], bq[:, 0:1])

        def qproj_half_d(ic, half, after=None):
            # one [128,256] half of the deferred Q^T projection chain
            qs = slice(ics(ic).start + 256 * half, ics(ic).start + 256 * (half + 1))
            ps = mp.tile([P, 256], f32, tag="mp", name=f"qpd_{ic}_{half}")
            for k in range(KT_TILES):
                mm = nc.tensor.matmul(ps[:], wqt[:, k, :], xt[:, k, qs],
                                      start=(k == 0), stop=(k == KT_TILES - 1))
                if after is not None and k == 0:
                    _add_dep_helper(mm.ins, after.ins, sync=False,
                                    reason="defer qproj behind attention")
            nc.vector.tensor_scalar_add(QT[:, qs], ps[:], bq[:, 0:1])

        def kproj_unit(jc):
            # K^T[:, jc*512:(jc+1)*512]
            ps = mp.tile([P, 512], f32, tag="mp", name=f"kp_{jc}")
            for k in range(KT_TILES):
                nc.tensor.matmul(ps[:], wkt[:, k, :], xt[:, k, ics(jc)],
                                 start=(k == 0), stop=(k == KT_TILES - 1))
            nc.vector.tensor_scalar_add(KT[:, ics(jc)], ps[:], bk[:, 0:1])

        def vproj_unit(jt):
            # V[jt] (natural layout) + split into the two per-head Vp slabs
            ps = op.tile([P, P], f32, tag="o", name=f"vp_{jt}")
            for k in range(KT_TILES):
                nc.tensor.matmul(ps[:], xt[:, k, jts(jt)], wvt[:, k, :],
                                 start=(k == 0), stop=(k == KT_TILES - 1))
            nc.vector.tensor_copy(Vp[:, jt, 0:64], ps[:, 0:64])
            nc.vector.tensor_copy(Vp[:, jt, 65:129], ps[:, 64:128])

        def kproj_unit_s(jc):
            # K^T chunk on an s-pool slot (prefix only: runs parallel to the
            # qproj on the mp bank)
            ps = sp.tile([P, 512], f32, tag="s", name=f"kps_{jc}")
            for k in range(KT_TILES):
                nc.tensor.matmul(ps[:], wkt[:, k, :], xt[:, k, ics(jc)],
                                 start=(k == 0), stop=(k == KT_TILES - 1))
            nc.vector.tensor_scalar_add(KT[:, ics(jc)], ps[:], bk[:, 0:1])

        # upfront: Q chunk 0 on mp, K chunks 0+1 on the two s-pool slots
        qproj_unit(0)
        kproj_unit_s(0)
        kproj_unit_s(1)

        # ---- attention (software-pipelined emission: S/exp of step t, PV of
        # ready pending steps; ScalarE tiles are PV-ready after 1 iteration,
        # DVE-route tiles after DVE_LAG (their P goes DVE poly -> gpsimd
        # square -> gpsimd square).  finalize is staged as in the baseline:
        # DVE-only work right after the last PV; PE work several iterations
        # later so the PE FIFO never waits on the reciprocal. ----
        def finalize_a(ic, o0, o1):
            # PSUM evict + bf16 cast of the denominator rows (row 64); the
            # bf16 rows make the rb broadcast matmuls run at bf16 rate
            # (fp32 matmuls are ~5x more PE time).  In the drain tail the
            # h1 copies go to the otherwise-idle ScalarE so the four copies
            # run two-abreast.
            oss = []
            for i, o in enumerate((o0, o1)):
                os_ = rrp.tile([65, 512], f32, tag="os", name=f"os_{ic}_{i}")
                dr = drp.tile([65, 512], dt_pv, tag="dr", name=f"dr_{ic}_{i}")
                if in_tail and i == 1:
                    nc.scalar.copy(os_[:], o[:])
                    nc.scalar.copy(dr[64:65, :], o[64:65, :])
                else:
                    nc.vector.tensor_copy(os_[:], o[:])
                    nc.vector.tensor_copy(dr[64:65, :], o[64:65, :])
                oss.append((os_, dr))
            return oss

        def norm_quantum(ic, st, h, half, after=None):
            # normalize one head/half: OT[:, slice] = os[0:64] * (1/r) via PE
            # outer-product of the (bf16) denominator (no recip dep in PE
            # FIFO) then reciprocal+mul on DVE.
            os_, dr = st[h]
            OTt = (OT0, OT1)[h]
            ls = slice(256 * half, 256 * (half + 1))
            hs = slice(ics(ic).start + 256 * half, ics(ic).start + 256 * (half + 1))
            pool_h = mp if h == 0 else op
            rb = pool_h.tile([HD, 256], f32, tag="mp" if h == 0 else "o",
                             name=f"rb_{ic}_{h}_{half}")
            mm = nc.tensor.matmul(rb[:], ones[64:65, :], dr[64:65, ls],
                                  start=True, stop=True, tile_position=(64, 0))
            if after is not None:
                _add_dep_helper(mm.ins, after.ins, sync=False,
                                reason="defer finalize rb behind attention")
            rbs = rbp.tile([HD, 256], f32, tag="rbs", name=f"rbs_{ic}_{h}_{half}")
            nc.vector.reciprocal_approx_fast(out=rbs[:], in_=rb[:])
            nc.vector.tensor_mul(OTt[:, hs], os_[0:64, ls], rbs[:])

        def yproj_quantum(ic, mt, idx, after=None, tail=False):
            # one [128,512] slice of the partial output projection; tail
            # quanta alternate between the mp and (now idle) o-pool banks
            hs = ics(ic)
            pool_y = op if (tail and mt % 2 == 1) else mp
            yps = pool_y.tile([P, 512], f32,
                              tag="o" if (tail and mt % 2 == 1) else "mp",
                              name=f"yp_{ic}_{mt}")
            mm = nc.tensor.matmul(yps[:], wot[:, 0, mts(mt)], OT0[:, hs],
                                  start=True, stop=False)
            if after is not None:
                _add_dep_helper(mm.ins, after.ins, sync=False,
                                reason="defer finalize yproj behind attention")
            nc.tensor.matmul(yps[:], wot[:, 1, mts(mt)], OT1[:, hs],
                             start=False, stop=True)
            ye = yep.tile([P, 512], f32, tag="ye", name=f"ye_{ic}_{mt}")
            if tail:
                nc.scalar.copy(ye[:], yps[:])   # ScalarE is idle in the tail
            else:
                nc.vector.tensor_copy(ye[:], yps[:])
            nc.sync.dma_start(yp_d[mt, :, hs], ye[:])

        # quantum schedule within the NEXT chunk: (jt, fn(args))
        NORM_SCHED = [(2, (0, 0)), (4, (1, 0)), (5, (0, 1)), (7, (1, 1))]
        YP_SCHED = {9: 0, 11: 1, 13: 2, 15: 3}

        def emit_exp(s, ic, jt):
            """Emit the exp of s -> P tile; returns (p_tile, ready_lag).

            DVE route runs per 512-col half (one head each) so the
            DVE->gpsimd->gpsimd chain latency per consumed half is short."""
            p = pp.tile([P, 1024], dt_pv, tag="p")
            route = DVE_JTS_LAST if ic == ICH - 1 else DVE_JTS
            if jt in route:
                # per-half chains: gpsimd starts squaring h0 while the DVE
                # polys h1, so P-h0 lands well before its lag-4 PV deadline
                for h in range(2):
                    cs = slice(512 * h, 512 * (h + 1))
                    ey = eyp.tile([P, 512], f32, tag="ey", name=f"ey_{ic}_{jt}_{h}")
                    nc.vector._custom_dve(exp_op, out=ey[:], in0=s[:, cs],
                                          s0=EC2, s1=EC3, imm2=EC4)
                    ez = ezp.tile([P, 512], f32, tag="ez", name=f"ez_{ic}_{jt}_{h}")
                    nc.gpsimd.tensor_mul(ez[:], ey[:], ey[:])
                    nc.gpsimd.tensor_tensor(p[:, cs], ez[:], ez[:], mult)
                return p, DVE_LAG, True
            nc.scalar.activation(p[:], s[:], Exp, scale=4.0)
            return p, SC_LAG, False

        otiles = {}
        in_tail = False
        issued0 = {}         # ic -> PV h0 matmuls issued
        issued1 = {}         # ic -> PV h1 matmuls issued
        pend = []            # list of [p_tile, ic, jt, ready_g]
        pend_h1 = []         # deferred first-of-ic h1 halves (o-bank WAR slack)
        pend_b = None        # (ic, stage-a state) awaiting finalize_b
        g = 0

        def issue_pv_h1(ptile, pic, pjt):
            nonlocal pend_b
            o0, o1 = otiles[pic]
            nc.tensor.matmul(o1[:], Vp[:, pjt, 65:130], ptile[:, 512:1024],
                             start=(issued1[pic] == 0),
                             stop=(issued1[pic] == JT - 1))
            issued1[pic] += 1
            if issued0[pic] == JT and issued1[pic] == JT:
                pend_b = (pic, finalize_a(pic, o0, o1))

        def issue_pv(ptile, pic, pjt, defer_h1=False):
            # h0 immediately; h1 deferred one iteration for the ic's FIRST
            # pair (o-bank WAR slack vs the previous ic's o0 eviction) and
            # for DVE-route tiles (their h1 half lands one iteration later)
            o0, o1 = otiles[pic]
            nc.tensor.matmul(o0[:], Vp[:, pjt, 0:65], ptile[:, 0:512],
                             start=(issued0[pic] == 0),
                             stop=(issued0[pic] == JT - 1))
            issued0[pic] += 1
            if defer_h1 or (issued1[pic] == 0 and issued0[pic] == 1):
                pend_h1.append((ptile, pic, pjt))
            else:
                issue_pv_h1(ptile, pic, pjt)

        for ic in range(ICH):
            otiles[ic] = (op.tile([65, 512], f32, tag="o", name=f"o0_{ic}"),
                          op.tile([65, 512], f32, tag="o", name=f"o1_{ic}"))
            issued0[ic] = 0
            issued1[ic] = 0
            for jt in range(JT):
                s = sp.tile([P, 1024], f32, tag="s")
                last_s = nc.tensor.matmul(s[:, 0:512], KT[0:64, jts(jt)],
                                          QT[0:64, ics(ic)],
                                          start=True, stop=True,
                                          tile_position=(0, 0))
                nc.tensor.matmul(s[:, 512:1024], KT[64:128, jts(jt)],
                                 QT[64:128, ics(ic)],
                                 start=True, stop=True, tile_position=(64, 0))
                p, lag, isdve = emit_exp(s, ic, jt)
                pend.append([p, ic, jt, g + lag, isdve])
                # deferred h1 halves first, then all PVs ready at this point
                # (issue order is the PSUM accumulation order)
                for ent in pend_h1[:]:
                    pend_h1.remove(ent)
                    issue_pv_h1(*ent)
                for ent in [e for e in pend if e[3] <= g]:
                    pend.remove(ent)
                    issue_pv(ent[0], ent[1], ent[2], defer_h1=ent[4])
                # deferred proj + finalize stages, spread across the loop
                if ic == 0:
                    if jt == 0:
                        vproj_unit(0)
                        vproj_unit(1)
                    elif jt <= JT - 2:
                        vproj_unit(jt + 1)
                    if jt < 24 and jt % 4 == 0:
                        kproj_unit(2 + jt // 4)
                if pend_b is not None:
                    bic, st = pend_b
                    for sjt, (h, half) in NORM_SCHED:
                        if jt == sjt:
                            norm_quantum(bic, st, h, half, after=last_s)
                    if jt in YP_SCHED:
                        mt = YP_SCHED[jt]
                        yproj_quantum(bic, mt, mt, after=last_s)
                        if mt == 3:
                            pend_b = None
                if jt in (19, 21) and ic + 1 < ICH:
                    qproj_half_d(ic + 1, (jt - 19) // 2, after=last_s)
                g += 1
        # drain the pipeline tail
        in_tail = True
        for ent in pend_h1[:]:
            pend_h1.remove(ent)
            issue_pv_h1(*ent)
        for ent in sorted(pend, key=lambda e: e[3]):
            issue_pv(ent[0], ent[1], ent[2])
        pic, st = pend_b
        for _, (h, half) in NORM_SCHED:
            norm_quantum(pic, st, h, half)
        for mt in range(4):
            yproj_quantum(pic, mt, mt, tail=True)

    nc.compile()
    _BUILD_CACHE[mode] = nc
    return nc


def _prep_inputs(x, qkv_w, qkv_b, out_w, mode):
    """Per-core input maps. Core c: batch c//4, head-pair c%4."""
    if mode == "bf16":
        dt_qk = np.dtype(ml_dtypes.bfloat16)
        dt_pv = dt_qk
    elif mode == "mixed":
        dt_qk = np.dtype(np.float32)
        dt_pv = np.dtype(ml_dtypes.bfloat16)
    else:
        dt_qk = np.dtype(np.float32)
        dt_pv = dt_qk

    x = np.asarray(x, np.float32)
    qkv_w = np.asarray(qkv_w, np.float32)
    qkv_b = np.asarray(qkv_b, np.float32)
    out_w = np.asarray(out_w, np.float32)

    xts = []
    for b in range(B):
        xt = np.ascontiguousarray(x[b].T).reshape(KT_TILES, P, N)
        xts.append(xt.astype(dt_qk))

    # q scale: 1/sqrt(hd)=0.125 plus an extra exact 1/4 so PSUM holds S/4
    # (ScalarE exp uses scale=4; the DVE poly route consumes S/4 directly)
    qs = 0.125 * 0.25
    in_maps = []
    for c in range(NCORES):
        b, m = divmod(c, 4)
        rs = slice(P * m, P * (m + 1))
        wq = (qs * qkv_w[0:D][rs]).T.reshape(KT_TILES, P, P)
        wk = qkv_w[D:2 * D][rs].T.reshape(KT_TILES, P, P)
        wv = qkv_w[2 * D:3 * D][rs].T.reshape(KT_TILES, P, P)
        wo = np.ascontiguousarray(out_w[:, rs].T).reshape(2, HD, D)
        in_maps.append({
            "xt": xts[b],
            "wqt": np.ascontiguousarray(wq).astype(dt_qk),
            "wkt": np.ascontiguousarray(wk).astype(dt_qk),
            "wvt": np.ascontiguousarray(wv).astype(dt_qk),
            "wot": wo.astype(dt_pv),
            "bq": (qs * qkv_b[0:D][rs]).reshape(P, 1).astype(np.float32),
            "bk": qkv_b[D:2 * D][rs].reshape(P, 1).astype(np.float32),
        })
    return in_maps


def _gather(results, qkv_b, out_w, out_b):
    # y[b] = (sum over the batch's 4 cores of yp)^T + out_w @ bv + out_b
    bias_vec = out_w.astype(np.float32) @ np.asarray(qkv_b, np.float32)[2 * D:3 * D] \
        + np.asarray(out_b, np.float32)
    y = np.empty((B, N, D), np.float32)
    for b in range(B):
        acc = np.zeros((D, N), np.float32)
        for m in range(4):
            acc += results[4 * b + m]["yp"].reshape(D, N)
        y[b] = acc.T + bias_vec
    return y


def _run(inputs, trace=False, tmpdir=None):
    from concourse.bass_utils import run_bass_kernel_spmd

    nc = _build(MODE)
    in_maps = _prep_inputs(inputs["x"], inputs["qkv_w"], inputs["qkv_b"],
                           inputs["out_w"], MODE)
    kw = {}
    if trace:
        kw = dict(trace=True, tmpdir=tmpdir)
    res = run_bass_kernel_spmd(nc, in_maps, core_ids=list(range(NCORES)), **kw)
    y = _gather(res.results, inputs["qkv_b"], inputs["out_w"], inputs["out_b"])
    return y, res


def kernel(x, qkv_w, qkv_b, out_w, out_b):
    y, _ = _run(dict(x=x, qkv_w=qkv_w, qkv_b=qkv_b, out_w=out_w, out_b=out_b))
    return y
